# revision 1
# baseline (speedup 1.0000x reference)
"""Butterfly (10-stage, n=1024) as a dense composed matmul on 8 TRN2 cores.

Strategy:
  - Host: compose the 10 butterfly stage matrices into one dense W
    (1024x1024, f64 accumulate -> f32). out = x @ W^T + bias.
  - Host: pack x into PE-friendly transposed tiles so every DMA is a
    contiguous 512KB read with 4KB partition lines:
        xt[tile][c'][j][b] = x[128*tile + b, 128*j + c']
  - Device (per core, 4096 rows = 32 tiles): for each tile, 16
    accumulating matmuls (lhsT = xt chunk [c'=128, b=128] stationary,
    rhs = W^T chunk [c'=128, n=512] moving, fp32r dtype -> 1 cycle/row),
    then DVE adds bias (replicated across partitions) while moving
    PSUM->SBUF, then DMA out (contiguous 512KB).
  - Data-parallel over batch: core k handles rows [4096k, 4096(k+1)).

Variants:
  - "f32r": float32r operands (~13-bit mantissa), f32 output. ~2e-4 rel err.
  - "bf16": bf16 operands and bf16 output; halves DMA traffic. ~3e-3 rel err.
  - "dma":  DMA in/out only, no compute (perf probe).
"""

import numpy as np
import ml_dtypes

import concourse.bass as bass
import concourse.bacc as bacc
import concourse.mybir as mybir
from concourse.tile import TileContext
from concourse.bass_utils import run_bass_kernel_spmd

N_CORES = 8
BATCH = 32768
NPOS = 1024
NSTAGE = 10
P = 128
NCHUNK = NPOS // P  # 8
TILES_PER_CORE = BATCH // N_CORES // P  # 32

VARIANT = "f32r"


def _compose_w(twiddle: np.ndarray) -> np.ndarray:
    """Compose the butterfly stages into M_id[c, n] = W[n, c] (= W^T).

    Applies the reference butterfly to the identity matrix in float64.
    Row c of the result is B @ e_c, i.e. column c of the composed W.
    """
    tw = np.asarray(twiddle, dtype=np.float64)  # (1, 10, 512, 2, 2)
    n = NPOS
    out = np.eye(n, dtype=np.float64).reshape(n, 1, n)
    for idx in range(NSTAGE):
        stride = 1 << idx
        nb = n // (2 * stride)
        t = tw[:, idx].reshape(1, nb, stride, 2, 2).transpose(0, 1, 3, 4, 2)
        o = out.reshape(n, 1, nb, 1, 2, stride)
        out = (t * o).sum(axis=4).reshape(n, 1, n)
    return out.reshape(n, n)  # [c, n]


def _build_nc(variant: str = VARIANT, repeats: int = 1) -> bass.Bass:
    nc = bacc.Bacc()
    f32 = mybir.dt.float32

    if variant == "bf16":
        in_dt = mybir.dt.bfloat16
        out_dt = mybir.dt.bfloat16
    else:
        in_dt = mybir.dt.float32r
        out_dt = f32

    xt = nc.declare_dram_parameter(
        "xt", [TILES_PER_CORE, P, NCHUNK, P], in_dt, isOutput=False
    )
    w = nc.declare_dram_parameter("w", [P, NCHUNK, NPOS], in_dt, isOutput=False)
    bias = nc.declare_dram_parameter("bias", [P, NPOS], f32, isOutput=False)
    out = nc.declare_dram_parameter(
        "out", [TILES_PER_CORE, P, NPOS], out_dt, isOutput=True
    )

    with TileContext(nc) as tc:
        with (
            tc.tile_pool(name="const", bufs=1) as cpool,
            tc.tile_pool(name="xtp", bufs=3) as xpool,
            tc.tile_pool(name="outp", bufs=3) as opool,
            tc.tile_pool(name="ps", bufs=4, space="PSUM") as pspool,
        ):
            w_sb = cpool.tile([P, NCHUNK, NPOS], in_dt)
            nc.sync.dma_start(out=w_sb[:], in_=w[:])
            b_sb = cpool.tile([P, NPOS], f32)
            nc.sync.dma_start(out=b_sb[:], in_=bias[:])

            for _rep in range(repeats):
                for t in range(TILES_PER_CORE):
                    xt_sb = xpool.tile([P, NCHUNK, P], in_dt)
                    nc.sync.dma_start(out=xt_sb[:], in_=xt[t])
                    o_sb = opool.tile([P, NPOS], out_dt)
                    if variant != "dma":
                        for nh in range(2):
                            ns = nh * 512
                            ps = pspool.tile([P, 512], f32)
                            for j in range(NCHUNK):
                                nc.tensor.matmul(
                                    ps[:],
                                    lhsT=xt_sb[:, j, :],
                                    rhs=w_sb[:, j, ns : ns + 512],
                                    start=(j == 0),
                                    stop=(j == NCHUNK - 1),
                                )
                            nc.vector.tensor_add(
                                out=o_sb[:, ns : ns + 512],
                                in0=ps[:],
                                in1=b_sb[:, ns : ns + 512],
                            )
                    if variant == "dma":
                        src = xt_sb[:].rearrange("p a b -> p (a b)").bitcast(out_dt)
                        nc.sync.dma_start(out=out[t], in_=src)
                    else:
                        nc.sync.dma_start(out=out[t], in_=o_sb[:])
    nc.compile()
    return nc


def _pack_inputs(x, twiddle, bias, variant: str = VARIANT):
    x = np.asarray(x, dtype=np.float32)
    bias = np.asarray(bias, dtype=np.float32)

    m_id = _compose_w(twiddle).astype(np.float32)  # [c, n] = W^T
    w_packed = np.ascontiguousarray(
        m_id.reshape(NCHUNK, P, NPOS).transpose(1, 0, 2)
    )  # [c', j, n]
    bias_rep = np.ascontiguousarray(np.broadcast_to(bias, (P, NPOS)))

    # [ntile, c', j, b] with ntile = 256 global tiles of 128 rows
    xt_all = np.ascontiguousarray(
        x.reshape(BATCH // P, P, NCHUNK, P).transpose(0, 3, 2, 1)
    )
    if variant == "bf16":
        xt_all = xt_all.astype(ml_dtypes.bfloat16)
        w_packed = w_packed.astype(ml_dtypes.bfloat16)
    return xt_all, w_packed, bias_rep


def kernel(x, twiddle, bias, _variant: str = "2lvl", _repeats: int = 1):
    """Harness entry point: full inputs in, full output out.

    Default path: two-level butterfly factorization (stages 0-6 as
    col-tiled block-diagonal bf16 matmuls, stages 7-9 as f32r matmuls in
    position-major space), f32 output, host re-transposes. Measured
    ~85-98us/pass on 8 cores, max rel err ~2.9e-3.
    Fallback _variant="f32r": dense composed-W f32r kernel,
    ~100-150us/pass, max rel err ~2e-4.
    """
    if _variant == "2lvl":
        return kernel_2lvl(x, twiddle, bias, out_bf16=False, _repeats=_repeats)
    xt_all, w_packed, bias_rep = _pack_inputs(x, twiddle, bias, _variant)

    nc = _build_nc(variant=_variant, repeats=_repeats)
    in_maps = [
        {
            "xt": xt_all[k * TILES_PER_CORE : (k + 1) * TILES_PER_CORE],
            "w": w_packed,
            "bias": bias_rep,
        }
        for k in range(N_CORES)
    ]
    res = run_bass_kernel_spmd(nc, in_maps, list(range(N_CORES)))

    out = np.concatenate(
        [np.asarray(r["out"]).reshape(-1, NPOS) for r in res.results], axis=0
    ).astype(np.float32)
    return out


# ---------------------------------------------------------------------------
# Two-level factorization: stages 0-6 (block-diag, col-tiled bf16 matmuls)
# then stages 7-9 (16 accumulating f32r matmuls), position-major orientation.
# Output is produced transposed ([pos, batch]); host re-transposes.
# ---------------------------------------------------------------------------

SBT_PER_CORE = 8  # super-tiles of 512 batch rows per core


def _apply_stages(tw, v, stages):
    b, n = v.shape
    out = v.reshape(b, 1, n)
    tw = np.asarray(tw, dtype=np.float64)
    for idx in stages:
        stride = 1 << idx
        nb = n // (2 * stride)
        t = tw[:, idx].reshape(1, nb, stride, 2, 2).transpose(0, 1, 3, 4, 2)
        o = out.reshape(b, 1, nb, 1, 2, stride)
        out = (t * o).sum(axis=4).reshape(b, 1, n)
    return out.reshape(b, n)


def _pack_2lvl(x, twiddle, bias, out_bf16: bool):
    x = np.asarray(x, dtype=np.float32)
    bias = np.asarray(bias, dtype=np.float64)
    n = NPOS
    I = np.eye(n)
    C_full = _apply_stages(twiddle, I, range(0, 7)).T  # [p, c]
    H = _apply_stages(twiddle, I, range(7, 10)).T      # [p', p]

    ca = np.empty((128, 8, 4, 32), np.float32)  # [c, k, S, m]
    for k in range(8):
        blk = C_full[128 * k : 128 * k + 128, 128 * k : 128 * k + 128]
        for S in range(4):
            ca[:, k, S, :] = blk[32 * S : 32 * S + 32, :].T
    ca = ca.astype(ml_dtypes.bfloat16)

    hb = np.empty((128, 4, 2, 2, 128), np.float32)  # [q, S, h, z, m]
    bt = np.empty((128, 8), np.float32)             # [q, 2S+h]
    for S in range(4):
        for h in range(2):
            rows_m = np.array(
                [128 * (4 * h + j) + 32 * S + s2 for j in range(4) for s2 in range(32)]
            )
            for z in range(2):
                cols_q = np.array(
                    [128 * (4 * z + k) + 32 * S + s for k in range(4) for s in range(32)]
                )
                hb[:, S, h, z, :] = H[np.ix_(rows_m, cols_q)].T
            bt[:, 2 * S + h] = bias[rows_m]
    bt = bt.astype(np.float32)

    # xt: [ncores, sbt, c', j, b] bf16
    xt = np.ascontiguousarray(
        x.reshape(N_CORES, SBT_PER_CORE, 512, NCHUNK, P).transpose(0, 1, 4, 3, 2)
    ).astype(ml_dtypes.bfloat16)
    return xt, ca, hb, bt


def _unpack_2lvl(core_outs):
    # core out: [sbt=8, S=4, h=2, m=128, b=512] -> [4096, 1024]
    parts = []
    for o in core_outs:
        arr = np.asarray(o).astype(np.float32)
        arr = arr.reshape(8, 4, 2, 4, 32, 512).transpose(0, 5, 2, 3, 1, 4)
        parts.append(arr.reshape(4096, 1024))
    return np.concatenate(parts, axis=0)


def _build_2lvl(out_bf16: bool, repeats: int = 1, xtp_bufs: int = 3, zrp_bufs: int = 3, outp_bufs: int = 6) -> bass.Bass:
    nc = bacc.Bacc()
    f32 = mybir.dt.float32
    f32r = mybir.dt.float32r
    bf16 = mybir.dt.bfloat16
    out_dt = bf16 if out_bf16 else f32

    xt = nc.declare_dram_parameter("xt", [SBT_PER_CORE, P, NCHUNK, 512], bf16, isOutput=False)
    ca = nc.declare_dram_parameter("ca", [P, 8, 4, 32], bf16, isOutput=False)
    hb = nc.declare_dram_parameter("hb", [P, 4, 2, 2, P], f32r, isOutput=False)
    bt = nc.declare_dram_parameter("bt", [P, 8], f32, isOutput=False)
    out = nc.declare_dram_parameter(
        "out", [SBT_PER_CORE, 4, 2, P, 512], out_dt, isOutput=True
    )

    with TileContext(nc) as tc:
        with (
            tc.tile_pool(name="const", bufs=1) as cpool,
            tc.tile_pool(name="xtp", bufs=xtp_bufs) as xpool,
            tc.tile_pool(name="zrp", bufs=zrp_bufs) as zrp,
            tc.tile_pool(name="outp", bufs=outp_bufs) as opool,
            tc.tile_pool(name="psA", bufs=2, space="PSUM") as psA,
            tc.tile_pool(name="psO", bufs=4, space="PSUM") as psO,
        ):
            ca_sb = cpool.tile([P, 8, 4, 32], bf16)
            nc.sync.dma_start(out=ca_sb[:], in_=ca[:])
            hb_sb = cpool.tile([P, 4, 2, 2, P], f32r)
            nc.sync.dma_start(out=hb_sb[:], in_=hb[:])
            bt_sb = cpool.tile([P, 8], f32)
            nc.sync.dma_start(out=bt_sb[:], in_=bt[:])

            for _rep in range(repeats):
                for sbt in range(SBT_PER_CORE):
                    xt_sb = xpool.tile([P, NCHUNK, 512], bf16)
                    nc.sync.dma_start(out=xt_sb[:], in_=xt[sbt])
                    for S in range(4):
                        zA = psA.tile([P, 512], f32, tag="zA")
                        zB = psA.tile([P, 512], f32, tag="zB")
                        for kk in range(4):
                            nc.tensor.matmul(
                                zA[32 * kk : 32 * kk + 32, :],
                                lhsT=ca_sb[:, kk, S, :],
                                rhs=xt_sb[:, kk, :],
                                start=True, stop=True,
                                tile_position=(0, 32 * kk),
                            )
                        for kk in range(4):
                            nc.tensor.matmul(
                                zB[32 * kk : 32 * kk + 32, :],
                                lhsT=ca_sb[:, 4 + kk, S, :],
                                rhs=xt_sb[:, 4 + kk, :],
                                start=True, stop=True,
                                tile_position=(0, 32 * kk),
                            )
                        zAr = zrp.tile([P, 512], f32r, tag="zAr")
                        nc.scalar.copy(out=zAr[:], in_=zA[:])
                        zBr = zrp.tile([P, 512], f32r, tag="zBr")
                        nc.scalar.copy(out=zBr[:], in_=zB[:])
                        for h in range(2):
                            po = psO.tile([P, 512], f32)
                            nc.tensor.matmul(
                                po[:], lhsT=hb_sb[:, S, h, 0, :], rhs=zAr[:],
                                start=True, stop=False,
                            )
                            nc.tensor.matmul(
                                po[:], lhsT=hb_sb[:, S, h, 1, :], rhs=zBr[:],
                                start=False, stop=True,
                            )
                            o_sb = opool.tile([P, 512], out_dt)
                            nc.vector.tensor_scalar_add(
                                out=o_sb[:], in0=po[:],
                                scalar1=bt_sb[:, 2 * S + h : 2 * S + h + 1],
                            )
                            nc.sync.dma_start(out=out[sbt, S, h], in_=o_sb[:])
    nc.compile()
    return nc


def kernel_2lvl(x, twiddle, bias, out_bf16=False, _repeats=1):
    xt, ca, hb, bt = _pack_2lvl(x, twiddle, bias, out_bf16)
    nc = _build_2lvl(out_bf16, repeats=_repeats)
    in_maps = [
        {"xt": xt[k], "ca": ca, "hb": hb, "bt": bt} for k in range(N_CORES)
    ]
    res = run_bass_kernel_spmd(nc, in_maps, list(range(N_CORES)))
    return _unpack_2lvl([r["out"] for r in res.results])


# --- 2lvl v2: z-copies as bf16 on DVE, phase B bf16, bias via K=1 matmul ---

def _pack_2lvl_v2(x, twiddle, bias):
    xt, ca, hb, bt = _pack_2lvl(x, twiddle, bias, True)
    hb_bf = np.asarray(hb, np.float32).astype(ml_dtypes.bfloat16)
    # bias as [1, 8, 128]: bt2[0, 2S+h, m]
    bt2 = np.ascontiguousarray(np.asarray(bt, np.float32).T.reshape(1, 8, 128)).astype(
        ml_dtypes.bfloat16
    )
    return xt, ca, hb_bf, bt2


def _build_2lvl_v2(repeats: int = 1) -> bass.Bass:
    nc = bacc.Bacc()
    f32 = mybir.dt.float32
    bf16 = mybir.dt.bfloat16

    xt = nc.declare_dram_parameter("xt", [SBT_PER_CORE, P, NCHUNK, 512], bf16, isOutput=False)
    ca = nc.declare_dram_parameter("ca", [P, 8, 4, 32], bf16, isOutput=False)
    hb = nc.declare_dram_parameter("hb", [P, 4, 2, 2, P], bf16, isOutput=False)
    bt = nc.declare_dram_parameter("bt", [1, 8, P], bf16, isOutput=False)
    out = nc.declare_dram_parameter(
        "out", [SBT_PER_CORE, 4, 2, P, 512], bf16, isOutput=True
    )

    with TileContext(nc) as tc:
        with (
            tc.tile_pool(name="const", bufs=1) as cpool,
            tc.tile_pool(name="xtp", bufs=2) as xpool,
            tc.tile_pool(name="zrp", bufs=2) as zrp,
            tc.tile_pool(name="outp", bufs=4) as opool,
            tc.tile_pool(name="psA", bufs=2, space="PSUM") as psA,
            tc.tile_pool(name="psO", bufs=4, space="PSUM") as psO,
        ):
            ca_sb = cpool.tile([P, 8, 4, 32], bf16)
            nc.sync.dma_start(out=ca_sb[:], in_=ca[:])
            hb_sb = cpool.tile([P, 4, 2, 2, P], bf16)
            nc.sync.dma_start(out=hb_sb[:], in_=hb[:])
            bt_sb = cpool.tile([1, 8, P], bf16)
            nc.sync.dma_start(out=bt_sb[:], in_=bt[:])
            ones_sb = cpool.tile([1, 512], bf16)
            nc.vector.memset(ones_sb[:], 1.0)

            for _rep in range(repeats):
                for sbt in range(SBT_PER_CORE):
                    xt_sb = xpool.tile([P, NCHUNK, 512], bf16)
                    nc.sync.dma_start(out=xt_sb[:], in_=xt[sbt])
                    for S in range(4):
                        zA = psA.tile([P, 512], f32, tag="zA")
                        zB = psA.tile([P, 512], f32, tag="zB")
                        for kk in range(4):
                            nc.tensor.matmul(
                                zA[32 * kk : 32 * kk + 32, :],
                                lhsT=ca_sb[:, kk, S, :],
                                rhs=xt_sb[:, kk, :],
                                start=True, stop=True,
                                tile_position=(0, 32 * kk),
                            )
                        for kk in range(4):
                            nc.tensor.matmul(
                                zB[32 * kk : 32 * kk + 32, :],
                                lhsT=ca_sb[:, 4 + kk, S, :],
                                rhs=xt_sb[:, 4 + kk, :],
                                start=True, stop=True,
                                tile_position=(0, 32 * kk),
                            )
                        zAr = zrp.tile([P, 512], bf16, tag="zAr")
                        nc.vector.tensor_copy(out=zAr[:], in_=zA[:])
                        zBr = zrp.tile([P, 512], bf16, tag="zBr")
                        nc.vector.tensor_copy(out=zBr[:], in_=zB[:])
                        for h in range(2):
                            po = psO.tile([P, 512], f32)
                            nc.tensor.matmul(
                                po[:], lhsT=bt_sb[:, 2 * S + h, :], rhs=ones_sb[:],
                                start=True, stop=False,
                            )
                            nc.tensor.matmul(
                                po[:], lhsT=hb_sb[:, S, h, 0, :], rhs=zAr[:],
                                start=False, stop=False,
                            )
                            nc.tensor.matmul(
                                po[:], lhsT=hb_sb[:, S, h, 1, :], rhs=zBr[:],
                                start=False, stop=True,
                            )
                            o_sb = opool.tile([P, 512], bf16)
                            nc.vector.tensor_copy(out=o_sb[:], in_=po[:])
                            nc.sync.dma_start(out=out[sbt, S, h], in_=o_sb[:])
    nc.compile()
    return nc


def kernel_2lvl_v2(x, twiddle, bias, _repeats=1):
    xt, ca, hb, bt = _pack_2lvl_v2(x, twiddle, bias)
    nc = _build_2lvl_v2(repeats=_repeats)
    in_maps = [
        {"xt": xt[k], "ca": ca, "hb": hb, "bt": bt} for k in range(N_CORES)
    ]
    res = run_bass_kernel_spmd(nc, in_maps, list(range(N_CORES)))
    return _unpack_2lvl([r["out"] for r in res.results])


# --- 2lvl v3: bf16 out, bias as K=1 matmul on PE, out-copies split ACT/DVE ---

def _pack_2lvl_v3(x, twiddle, bias):
    xt, ca, hb, bt = _pack_2lvl(x, twiddle, bias, True)
    # bias as [1, 8, 128] bf16 for the K=1 matmul: bt2[0, 2S+h, m]
    bt2 = np.ascontiguousarray(np.asarray(bt, np.float32).T.reshape(1, 8, 128)).astype(
        ml_dtypes.bfloat16
    )
    return xt, ca, hb, bt2


def _build_2lvl_v3(repeats: int = 1) -> bass.Bass:
    nc = bacc.Bacc()
    f32 = mybir.dt.float32
    f32r = mybir.dt.float32r
    bf16 = mybir.dt.bfloat16

    xt = nc.declare_dram_parameter("xt", [SBT_PER_CORE, P, NCHUNK, 512], bf16, isOutput=False)
    ca = nc.declare_dram_parameter("ca", [P, 8, 4, 32], bf16, isOutput=False)
    hb = nc.declare_dram_parameter("hb", [P, 4, 2, 2, P], f32r, isOutput=False)
    bt = nc.declare_dram_parameter("bt", [1, 8, P], bf16, isOutput=False)
    out = nc.declare_dram_parameter(
        "out", [SBT_PER_CORE, 4, 2, P, 512], bf16, isOutput=True
    )

    with TileContext(nc) as tc:
        with (
            tc.tile_pool(name="const", bufs=1) as cpool,
            tc.tile_pool(name="xtp", bufs=2) as xpool,
            tc.tile_pool(name="zrp", bufs=2) as zrp,
            tc.tile_pool(name="outp", bufs=4) as opool,
            tc.tile_pool(name="psA", bufs=2, space="PSUM") as psA,
            tc.tile_pool(name="psO", bufs=4, space="PSUM") as psO,
        ):
            ca_sb = cpool.tile([P, 8, 4, 32], bf16)
            nc.sync.dma_start(out=ca_sb[:], in_=ca[:])
            hb_sb = cpool.tile([P, 4, 2, 2, P], f32r)
            nc.sync.dma_start(out=hb_sb[:], in_=hb[:])
            bt_sb = cpool.tile([1, 8, P], bf16)
            nc.sync.dma_start(out=bt_sb[:], in_=bt[:])
            ones_sb = cpool.tile([1, 512], bf16)
            nc.vector.memset(ones_sb[:], 1.0)

            for _rep in range(repeats):
                for sbt in range(SBT_PER_CORE):
                    xt_sb = xpool.tile([P, NCHUNK, 512], bf16)
                    nc.sync.dma_start(out=xt_sb[:], in_=xt[sbt])
                    for S in range(4):
                        zA = psA.tile([P, 512], f32, tag="zA")
                        zB = psA.tile([P, 512], f32, tag="zB")
                        for kk in range(4):
                            nc.tensor.matmul(
                                zA[32 * kk : 32 * kk + 32, :],
                                lhsT=ca_sb[:, kk, S, :],
                                rhs=xt_sb[:, kk, :],
                                start=True, stop=True,
                                tile_position=(0, 32 * kk),
                            )
                        for kk in range(4):
                            nc.tensor.matmul(
                                zB[32 * kk : 32 * kk + 32, :],
                                lhsT=ca_sb[:, 4 + kk, S, :],
                                rhs=xt_sb[:, 4 + kk, :],
                                start=True, stop=True,
                                tile_position=(0, 32 * kk),
                            )
                        zAr = zrp.tile([P, 512], f32r, tag="zAr")
                        nc.scalar.copy(out=zAr[:], in_=zA[:])
                        zBr = zrp.tile([P, 512], f32r, tag="zBr")
                        nc.scalar.copy(out=zBr[:], in_=zB[:])
                        for h in range(2):
                            po = psO.tile([P, 512], f32)
                            nc.tensor.matmul(
                                po[:], lhsT=bt_sb[:, 2 * S + h, :], rhs=ones_sb[:],
                                start=True, stop=False,
                            )
                            nc.tensor.matmul(
                                po[:], lhsT=hb_sb[:, S, h, 0, :], rhs=zAr[:],
                                start=False, stop=False,
                            )
                            nc.tensor.matmul(
                                po[:], lhsT=hb_sb[:, S, h, 1, :], rhs=zBr[:],
                                start=False, stop=True,
                            )
                            o_sb = opool.tile([P, 512], bf16)
                            if (2 * S + h) % 2 == 0:
                                nc.scalar.copy(out=o_sb[:], in_=po[:])
                            else:
                                nc.vector.tensor_copy(out=o_sb[:], in_=po[:])
                            nc.sync.dma_start(out=out[sbt, S, h], in_=o_sb[:])
    nc.compile()
    return nc


def kernel_2lvl_v3(x, twiddle, bias, _repeats=1):
    xt, ca, hb, bt = _pack_2lvl_v3(x, twiddle, bias)
    nc = _build_2lvl_v3(repeats=_repeats)
    in_maps = [
        {"xt": xt[k], "ca": ca, "hb": hb, "bt": bt} for k in range(N_CORES)
    ]
    res = run_bass_kernel_spmd(nc, in_maps, list(range(N_CORES)))
    return _unpack_2lvl([r["out"] for r in res.results])





# revision 6
# speedup vs baseline: 1.7697x; 1.7697x over previous
"""Butterfly (10-stage, n=1024) as a dense composed matmul on 8 TRN2 cores.

Strategy:
  - Host: compose the 10 butterfly stage matrices into one dense W
    (1024x1024, f64 accumulate -> f32). out = x @ W^T + bias.
  - Host: pack x into PE-friendly transposed tiles so every DMA is a
    contiguous 512KB read with 4KB partition lines:
        xt[tile][c'][j][b] = x[128*tile + b, 128*j + c']
  - Device (per core, 4096 rows = 32 tiles): for each tile, 16
    accumulating matmuls (lhsT = xt chunk [c'=128, b=128] stationary,
    rhs = W^T chunk [c'=128, n=512] moving, fp32r dtype -> 1 cycle/row),
    then DVE adds bias (replicated across partitions) while moving
    PSUM->SBUF, then DMA out (contiguous 512KB).
  - Data-parallel over batch: core k handles rows [4096k, 4096(k+1)).

Variants:
  - "f32r": float32r operands (~13-bit mantissa), f32 output. ~2e-4 rel err.
  - "bf16": bf16 operands and bf16 output; halves DMA traffic. ~3e-3 rel err.
  - "dma":  DMA in/out only, no compute (perf probe).
"""

import numpy as np
import ml_dtypes

import concourse.bass as bass
import concourse.bacc as bacc
import concourse.mybir as mybir
from concourse.tile import TileContext
from concourse.bass_utils import run_bass_kernel_spmd

N_CORES = 8
BATCH = 32768
NPOS = 1024
NSTAGE = 10
P = 128
NCHUNK = NPOS // P  # 8
TILES_PER_CORE = BATCH // N_CORES // P  # 32

VARIANT = "f32r"


def _compose_w(twiddle: np.ndarray) -> np.ndarray:
    """Compose the butterfly stages into M_id[c, n] = W[n, c] (= W^T).

    Applies the reference butterfly to the identity matrix in float64.
    Row c of the result is B @ e_c, i.e. column c of the composed W.
    """
    tw = np.asarray(twiddle, dtype=np.float64)  # (1, 10, 512, 2, 2)
    n = NPOS
    out = np.eye(n, dtype=np.float64).reshape(n, 1, n)
    for idx in range(NSTAGE):
        stride = 1 << idx
        nb = n // (2 * stride)
        t = tw[:, idx].reshape(1, nb, stride, 2, 2).transpose(0, 1, 3, 4, 2)
        o = out.reshape(n, 1, nb, 1, 2, stride)
        out = (t * o).sum(axis=4).reshape(n, 1, n)
    return out.reshape(n, n)  # [c, n]


def _build_nc(variant: str = VARIANT, repeats: int = 1) -> bass.Bass:
    nc = bacc.Bacc()
    f32 = mybir.dt.float32

    if variant == "bf16":
        in_dt = mybir.dt.bfloat16
        out_dt = mybir.dt.bfloat16
    else:
        in_dt = mybir.dt.float32r
        out_dt = f32

    xt = nc.declare_dram_parameter(
        "xt", [TILES_PER_CORE, P, NCHUNK, P], in_dt, isOutput=False
    )
    w = nc.declare_dram_parameter("w", [P, NCHUNK, NPOS], in_dt, isOutput=False)
    bias = nc.declare_dram_parameter("bias", [P, NPOS], f32, isOutput=False)
    out = nc.declare_dram_parameter(
        "out", [TILES_PER_CORE, P, NPOS], out_dt, isOutput=True
    )

    with TileContext(nc) as tc:
        with (
            tc.tile_pool(name="const", bufs=1) as cpool,
            tc.tile_pool(name="xtp", bufs=3) as xpool,
            tc.tile_pool(name="outp", bufs=3) as opool,
            tc.tile_pool(name="ps", bufs=4, space="PSUM") as pspool,
        ):
            w_sb = cpool.tile([P, NCHUNK, NPOS], in_dt)
            nc.sync.dma_start(out=w_sb[:], in_=w[:])
            b_sb = cpool.tile([P, NPOS], f32)
            nc.sync.dma_start(out=b_sb[:], in_=bias[:])

            for _rep in range(repeats):
                for t in range(TILES_PER_CORE):
                    xt_sb = xpool.tile([P, NCHUNK, P], in_dt)
                    nc.sync.dma_start(out=xt_sb[:], in_=xt[t])
                    o_sb = opool.tile([P, NPOS], out_dt)
                    if variant != "dma":
                        for nh in range(2):
                            ns = nh * 512
                            ps = pspool.tile([P, 512], f32)
                            for j in range(NCHUNK):
                                nc.tensor.matmul(
                                    ps[:],
                                    lhsT=xt_sb[:, j, :],
                                    rhs=w_sb[:, j, ns : ns + 512],
                                    start=(j == 0),
                                    stop=(j == NCHUNK - 1),
                                )
                            nc.vector.tensor_add(
                                out=o_sb[:, ns : ns + 512],
                                in0=ps[:],
                                in1=b_sb[:, ns : ns + 512],
                            )
                    if variant == "dma":
                        src = xt_sb[:].rearrange("p a b -> p (a b)").bitcast(out_dt)
                        nc.sync.dma_start(out=out[t], in_=src)
                    else:
                        nc.sync.dma_start(out=out[t], in_=o_sb[:])
    nc.compile()
    return nc


def _pack_inputs(x, twiddle, bias, variant: str = VARIANT):
    x = np.asarray(x, dtype=np.float32)
    bias = np.asarray(bias, dtype=np.float32)

    m_id = _compose_w(twiddle).astype(np.float32)  # [c, n] = W^T
    w_packed = np.ascontiguousarray(
        m_id.reshape(NCHUNK, P, NPOS).transpose(1, 0, 2)
    )  # [c', j, n]
    bias_rep = np.ascontiguousarray(np.broadcast_to(bias, (P, NPOS)))

    # [ntile, c', j, b] with ntile = 256 global tiles of 128 rows
    xt_all = np.ascontiguousarray(
        x.reshape(BATCH // P, P, NCHUNK, P).transpose(0, 3, 2, 1)
    )
    if variant == "bf16":
        xt_all = xt_all.astype(ml_dtypes.bfloat16)
        w_packed = w_packed.astype(ml_dtypes.bfloat16)
    return xt_all, w_packed, bias_rep


def kernel(x, twiddle, bias, _variant: str = "2lvl_v4c", _repeats: int = 1):
    """Harness entry point: full inputs in, full output out.

    Default path (2lvl_v4c): two-level butterfly factorization, all-bf16
    matmuls (stages 0-6 as col-tiled block-diagonal matmuls exploiting PE
    sub-array concurrency, stages 7-9 as K=128 matmuls in band-mixed z
    space), bias added on ACT, bf16 output. Input DMAs ride the SP HWDGE
    ring, output DMAs the ACT ring (issued right after the bias-adds that
    produce them) so transfers overlap compute. Measured ~41.6us/pass on
    8 cores (DMA floor ~39.7us for 8MiB in + 8MiB out bf16 at ~423GB/s),
    max rel err ~6.4e-3.
    Fallback _variant="2lvl": older f32-out pipeline, ~79us/pass.
    Fallback _variant="f32r": dense composed-W f32r kernel,
    ~100-150us/pass, max rel err ~2e-4.
    """
    if _variant == "2lvl_v4c":
        return kernel_2lvl_v4c(x, twiddle, bias, _repeats=_repeats)
    if _variant == "2lvl":
        return kernel_2lvl(x, twiddle, bias, out_bf16=False, _repeats=_repeats)
    xt_all, w_packed, bias_rep = _pack_inputs(x, twiddle, bias, _variant)

    nc = _build_nc(variant=_variant, repeats=_repeats)
    in_maps = [
        {
            "xt": xt_all[k * TILES_PER_CORE : (k + 1) * TILES_PER_CORE],
            "w": w_packed,
            "bias": bias_rep,
        }
        for k in range(N_CORES)
    ]
    res = run_bass_kernel_spmd(nc, in_maps, list(range(N_CORES)))

    out = np.concatenate(
        [np.asarray(r["out"]).reshape(-1, NPOS) for r in res.results], axis=0
    ).astype(np.float32)
    return out


# ---------------------------------------------------------------------------
# Two-level factorization: stages 0-6 (block-diag, col-tiled bf16 matmuls)
# then stages 7-9 (16 accumulating f32r matmuls), position-major orientation.
# Output is produced transposed ([pos, batch]); host re-transposes.
# ---------------------------------------------------------------------------

SBT_PER_CORE = 8  # super-tiles of 512 batch rows per core


def _apply_stages(tw, v, stages):
    b, n = v.shape
    out = v.reshape(b, 1, n)
    tw = np.asarray(tw, dtype=np.float64)
    for idx in stages:
        stride = 1 << idx
        nb = n // (2 * stride)
        t = tw[:, idx].reshape(1, nb, stride, 2, 2).transpose(0, 1, 3, 4, 2)
        o = out.reshape(b, 1, nb, 1, 2, stride)
        out = (t * o).sum(axis=4).reshape(b, 1, n)
    return out.reshape(b, n)


def _pack_2lvl(x, twiddle, bias, out_bf16: bool):
    x = np.asarray(x, dtype=np.float32)
    bias = np.asarray(bias, dtype=np.float64)
    n = NPOS
    I = np.eye(n)
    C_full = _apply_stages(twiddle, I, range(0, 7)).T  # [p, c]
    H = _apply_stages(twiddle, I, range(7, 10)).T      # [p', p]

    ca = np.empty((128, 8, 4, 32), np.float32)  # [c, k, S, m]
    for k in range(8):
        blk = C_full[128 * k : 128 * k + 128, 128 * k : 128 * k + 128]
        for S in range(4):
            ca[:, k, S, :] = blk[32 * S : 32 * S + 32, :].T
    ca = ca.astype(ml_dtypes.bfloat16)

    hb = np.empty((128, 4, 2, 2, 128), np.float32)  # [q, S, h, z, m]
    bt = np.empty((128, 8), np.float32)             # [q, 2S+h]
    for S in range(4):
        for h in range(2):
            rows_m = np.array(
                [128 * (4 * h + j) + 32 * S + s2 for j in range(4) for s2 in range(32)]
            )
            for z in range(2):
                cols_q = np.array(
                    [128 * (4 * z + k) + 32 * S + s for k in range(4) for s in range(32)]
                )
                hb[:, S, h, z, :] = H[np.ix_(rows_m, cols_q)].T
            bt[:, 2 * S + h] = bias[rows_m]
    bt = bt.astype(np.float32)

    # xt: [ncores, sbt, c', j, b] bf16
    xt = np.ascontiguousarray(
        x.reshape(N_CORES, SBT_PER_CORE, 512, NCHUNK, P).transpose(0, 1, 4, 3, 2)
    ).astype(ml_dtypes.bfloat16)
    return xt, ca, hb, bt


def _unpack_2lvl(core_outs):
    # core out: [sbt=8, S=4, h=2, m=128, b=512] -> [4096, 1024]
    parts = []
    for o in core_outs:
        arr = np.asarray(o).astype(np.float32)
        arr = arr.reshape(8, 4, 2, 4, 32, 512).transpose(0, 5, 2, 3, 1, 4)
        parts.append(arr.reshape(4096, 1024))
    return np.concatenate(parts, axis=0)


def _build_2lvl(out_bf16: bool, repeats: int = 1, xtp_bufs: int = 3, zrp_bufs: int = 3, outp_bufs: int = 6) -> bass.Bass:
    nc = bacc.Bacc()
    f32 = mybir.dt.float32
    f32r = mybir.dt.float32r
    bf16 = mybir.dt.bfloat16
    out_dt = bf16 if out_bf16 else f32

    xt = nc.declare_dram_parameter("xt", [SBT_PER_CORE, P, NCHUNK, 512], bf16, isOutput=False)
    ca = nc.declare_dram_parameter("ca", [P, 8, 4, 32], bf16, isOutput=False)
    hb = nc.declare_dram_parameter("hb", [P, 4, 2, 2, P], f32r, isOutput=False)
    bt = nc.declare_dram_parameter("bt", [P, 8], f32, isOutput=False)
    out = nc.declare_dram_parameter(
        "out", [SBT_PER_CORE, 4, 2, P, 512], out_dt, isOutput=True
    )

    with TileContext(nc) as tc:
        with (
            tc.tile_pool(name="const", bufs=1) as cpool,
            tc.tile_pool(name="xtp", bufs=xtp_bufs) as xpool,
            tc.tile_pool(name="zrp", bufs=zrp_bufs) as zrp,
            tc.tile_pool(name="outp", bufs=outp_bufs) as opool,
            tc.tile_pool(name="psA", bufs=2, space="PSUM") as psA,
            tc.tile_pool(name="psO", bufs=4, space="PSUM") as psO,
        ):
            ca_sb = cpool.tile([P, 8, 4, 32], bf16)
            nc.sync.dma_start(out=ca_sb[:], in_=ca[:])
            hb_sb = cpool.tile([P, 4, 2, 2, P], f32r)
            nc.sync.dma_start(out=hb_sb[:], in_=hb[:])
            bt_sb = cpool.tile([P, 8], f32)
            nc.sync.dma_start(out=bt_sb[:], in_=bt[:])

            for _rep in range(repeats):
                for sbt in range(SBT_PER_CORE):
                    xt_sb = xpool.tile([P, NCHUNK, 512], bf16)
                    nc.sync.dma_start(out=xt_sb[:], in_=xt[sbt])
                    for S in range(4):
                        zA = psA.tile([P, 512], f32, tag="zA")
                        zB = psA.tile([P, 512], f32, tag="zB")
                        for kk in range(4):
                            nc.tensor.matmul(
                                zA[32 * kk : 32 * kk + 32, :],
                                lhsT=ca_sb[:, kk, S, :],
                                rhs=xt_sb[:, kk, :],
                                start=True, stop=True,
                                tile_position=(0, 32 * kk),
                            )
                        for kk in range(4):
                            nc.tensor.matmul(
                                zB[32 * kk : 32 * kk + 32, :],
                                lhsT=ca_sb[:, 4 + kk, S, :],
                                rhs=xt_sb[:, 4 + kk, :],
                                start=True, stop=True,
                                tile_position=(0, 32 * kk),
                            )
                        zAr = zrp.tile([P, 512], f32r, tag="zAr")
                        nc.scalar.copy(out=zAr[:], in_=zA[:])
                        zBr = zrp.tile([P, 512], f32r, tag="zBr")
                        nc.scalar.copy(out=zBr[:], in_=zB[:])
                        for h in range(2):
                            po = psO.tile([P, 512], f32)
                            nc.tensor.matmul(
                                po[:], lhsT=hb_sb[:, S, h, 0, :], rhs=zAr[:],
                                start=True, stop=False,
                            )
                            nc.tensor.matmul(
                                po[:], lhsT=hb_sb[:, S, h, 1, :], rhs=zBr[:],
                                start=False, stop=True,
                            )
                            o_sb = opool.tile([P, 512], out_dt)
                            nc.vector.tensor_scalar_add(
                                out=o_sb[:], in0=po[:],
                                scalar1=bt_sb[:, 2 * S + h : 2 * S + h + 1],
                            )
                            nc.sync.dma_start(out=out[sbt, S, h], in_=o_sb[:])
    nc.compile()
    return nc


def kernel_2lvl(x, twiddle, bias, out_bf16=False, _repeats=1):
    xt, ca, hb, bt = _pack_2lvl(x, twiddle, bias, out_bf16)
    nc = _build_2lvl(out_bf16, repeats=_repeats)
    in_maps = [
        {"xt": xt[k], "ca": ca, "hb": hb, "bt": bt} for k in range(N_CORES)
    ]
    res = run_bass_kernel_spmd(nc, in_maps, list(range(N_CORES)))
    return _unpack_2lvl([r["out"] for r in res.results])


# --- 2lvl v2: z-copies as bf16 on DVE, phase B bf16, bias via K=1 matmul ---

def _pack_2lvl_v2(x, twiddle, bias):
    xt, ca, hb, bt = _pack_2lvl(x, twiddle, bias, True)
    hb_bf = np.asarray(hb, np.float32).astype(ml_dtypes.bfloat16)
    # bias as [1, 8, 128]: bt2[0, 2S+h, m]
    bt2 = np.ascontiguousarray(np.asarray(bt, np.float32).T.reshape(1, 8, 128)).astype(
        ml_dtypes.bfloat16
    )
    return xt, ca, hb_bf, bt2


def _build_2lvl_v2(repeats: int = 1) -> bass.Bass:
    nc = bacc.Bacc()
    f32 = mybir.dt.float32
    bf16 = mybir.dt.bfloat16

    xt = nc.declare_dram_parameter("xt", [SBT_PER_CORE, P, NCHUNK, 512], bf16, isOutput=False)
    ca = nc.declare_dram_parameter("ca", [P, 8, 4, 32], bf16, isOutput=False)
    hb = nc.declare_dram_parameter("hb", [P, 4, 2, 2, P], bf16, isOutput=False)
    bt = nc.declare_dram_parameter("bt", [1, 8, P], bf16, isOutput=False)
    out = nc.declare_dram_parameter(
        "out", [SBT_PER_CORE, 4, 2, P, 512], bf16, isOutput=True
    )

    with TileContext(nc) as tc:
        with (
            tc.tile_pool(name="const", bufs=1) as cpool,
            tc.tile_pool(name="xtp", bufs=2) as xpool,
            tc.tile_pool(name="zrp", bufs=2) as zrp,
            tc.tile_pool(name="outp", bufs=4) as opool,
            tc.tile_pool(name="psA", bufs=2, space="PSUM") as psA,
            tc.tile_pool(name="psO", bufs=4, space="PSUM") as psO,
        ):
            ca_sb = cpool.tile([P, 8, 4, 32], bf16)
            nc.sync.dma_start(out=ca_sb[:], in_=ca[:])
            hb_sb = cpool.tile([P, 4, 2, 2, P], bf16)
            nc.sync.dma_start(out=hb_sb[:], in_=hb[:])
            bt_sb = cpool.tile([1, 8, P], bf16)
            nc.sync.dma_start(out=bt_sb[:], in_=bt[:])
            ones_sb = cpool.tile([1, 512], bf16)
            nc.vector.memset(ones_sb[:], 1.0)

            for _rep in range(repeats):
                for sbt in range(SBT_PER_CORE):
                    xt_sb = xpool.tile([P, NCHUNK, 512], bf16)
                    nc.sync.dma_start(out=xt_sb[:], in_=xt[sbt])
                    for S in range(4):
                        zA = psA.tile([P, 512], f32, tag="zA")
                        zB = psA.tile([P, 512], f32, tag="zB")
                        for kk in range(4):
                            nc.tensor.matmul(
                                zA[32 * kk : 32 * kk + 32, :],
                                lhsT=ca_sb[:, kk, S, :],
                                rhs=xt_sb[:, kk, :],
                                start=True, stop=True,
                                tile_position=(0, 32 * kk),
                            )
                        for kk in range(4):
                            nc.tensor.matmul(
                                zB[32 * kk : 32 * kk + 32, :],
                                lhsT=ca_sb[:, 4 + kk, S, :],
                                rhs=xt_sb[:, 4 + kk, :],
                                start=True, stop=True,
                                tile_position=(0, 32 * kk),
                            )
                        zAr = zrp.tile([P, 512], bf16, tag="zAr")
                        nc.vector.tensor_copy(out=zAr[:], in_=zA[:])
                        zBr = zrp.tile([P, 512], bf16, tag="zBr")
                        nc.vector.tensor_copy(out=zBr[:], in_=zB[:])
                        for h in range(2):
                            po = psO.tile([P, 512], f32)
                            nc.tensor.matmul(
                                po[:], lhsT=bt_sb[:, 2 * S + h, :], rhs=ones_sb[:],
                                start=True, stop=False,
                            )
                            nc.tensor.matmul(
                                po[:], lhsT=hb_sb[:, S, h, 0, :], rhs=zAr[:],
                                start=False, stop=False,
                            )
                            nc.tensor.matmul(
                                po[:], lhsT=hb_sb[:, S, h, 1, :], rhs=zBr[:],
                                start=False, stop=True,
                            )
                            o_sb = opool.tile([P, 512], bf16)
                            nc.vector.tensor_copy(out=o_sb[:], in_=po[:])
                            nc.sync.dma_start(out=out[sbt, S, h], in_=o_sb[:])
    nc.compile()
    return nc


def kernel_2lvl_v2(x, twiddle, bias, _repeats=1):
    xt, ca, hb, bt = _pack_2lvl_v2(x, twiddle, bias)
    nc = _build_2lvl_v2(repeats=_repeats)
    in_maps = [
        {"xt": xt[k], "ca": ca, "hb": hb, "bt": bt} for k in range(N_CORES)
    ]
    res = run_bass_kernel_spmd(nc, in_maps, list(range(N_CORES)))
    return _unpack_2lvl([r["out"] for r in res.results])


# --- 2lvl v3: bf16 out, bias as K=1 matmul on PE, out-copies split ACT/DVE ---

def _pack_2lvl_v3(x, twiddle, bias):
    xt, ca, hb, bt = _pack_2lvl(x, twiddle, bias, True)
    # bias as [1, 8, 128] bf16 for the K=1 matmul: bt2[0, 2S+h, m]
    bt2 = np.ascontiguousarray(np.asarray(bt, np.float32).T.reshape(1, 8, 128)).astype(
        ml_dtypes.bfloat16
    )
    return xt, ca, hb, bt2


def _build_2lvl_v3(repeats: int = 1) -> bass.Bass:
    nc = bacc.Bacc()
    f32 = mybir.dt.float32
    f32r = mybir.dt.float32r
    bf16 = mybir.dt.bfloat16

    xt = nc.declare_dram_parameter("xt", [SBT_PER_CORE, P, NCHUNK, 512], bf16, isOutput=False)
    ca = nc.declare_dram_parameter("ca", [P, 8, 4, 32], bf16, isOutput=False)
    hb = nc.declare_dram_parameter("hb", [P, 4, 2, 2, P], f32r, isOutput=False)
    bt = nc.declare_dram_parameter("bt", [1, 8, P], bf16, isOutput=False)
    out = nc.declare_dram_parameter(
        "out", [SBT_PER_CORE, 4, 2, P, 512], bf16, isOutput=True
    )

    with TileContext(nc) as tc:
        with (
            tc.tile_pool(name="const", bufs=1) as cpool,
            tc.tile_pool(name="xtp", bufs=2) as xpool,
            tc.tile_pool(name="zrp", bufs=2) as zrp,
            tc.tile_pool(name="outp", bufs=4) as opool,
            tc.tile_pool(name="psA", bufs=2, space="PSUM") as psA,
            tc.tile_pool(name="psO", bufs=4, space="PSUM") as psO,
        ):
            ca_sb = cpool.tile([P, 8, 4, 32], bf16)
            nc.sync.dma_start(out=ca_sb[:], in_=ca[:])
            hb_sb = cpool.tile([P, 4, 2, 2, P], f32r)
            nc.sync.dma_start(out=hb_sb[:], in_=hb[:])
            bt_sb = cpool.tile([1, 8, P], bf16)
            nc.sync.dma_start(out=bt_sb[:], in_=bt[:])
            ones_sb = cpool.tile([1, 512], bf16)
            nc.vector.memset(ones_sb[:], 1.0)

            for _rep in range(repeats):
                for sbt in range(SBT_PER_CORE):
                    xt_sb = xpool.tile([P, NCHUNK, 512], bf16)
                    nc.sync.dma_start(out=xt_sb[:], in_=xt[sbt])
                    for S in range(4):
                        zA = psA.tile([P, 512], f32, tag="zA")
                        zB = psA.tile([P, 512], f32, tag="zB")
                        for kk in range(4):
                            nc.tensor.matmul(
                                zA[32 * kk : 32 * kk + 32, :],
                                lhsT=ca_sb[:, kk, S, :],
                                rhs=xt_sb[:, kk, :],
                                start=True, stop=True,
                                tile_position=(0, 32 * kk),
                            )
                        for kk in range(4):
                            nc.tensor.matmul(
                                zB[32 * kk : 32 * kk + 32, :],
                                lhsT=ca_sb[:, 4 + kk, S, :],
                                rhs=xt_sb[:, 4 + kk, :],
                                start=True, stop=True,
                                tile_position=(0, 32 * kk),
                            )
                        zAr = zrp.tile([P, 512], f32r, tag="zAr")
                        nc.scalar.copy(out=zAr[:], in_=zA[:])
                        zBr = zrp.tile([P, 512], f32r, tag="zBr")
                        nc.scalar.copy(out=zBr[:], in_=zB[:])
                        for h in range(2):
                            po = psO.tile([P, 512], f32)
                            nc.tensor.matmul(
                                po[:], lhsT=bt_sb[:, 2 * S + h, :], rhs=ones_sb[:],
                                start=True, stop=False,
                            )
                            nc.tensor.matmul(
                                po[:], lhsT=hb_sb[:, S, h, 0, :], rhs=zAr[:],
                                start=False, stop=False,
                            )
                            nc.tensor.matmul(
                                po[:], lhsT=hb_sb[:, S, h, 1, :], rhs=zBr[:],
                                start=False, stop=True,
                            )
                            o_sb = opool.tile([P, 512], bf16)
                            if (2 * S + h) % 2 == 0:
                                nc.scalar.copy(out=o_sb[:], in_=po[:])
                            else:
                                nc.vector.tensor_copy(out=o_sb[:], in_=po[:])
                            nc.sync.dma_start(out=out[sbt, S, h], in_=o_sb[:])
    nc.compile()
    return nc


def kernel_2lvl_v3(x, twiddle, bias, _repeats=1):
    xt, ca, hb, bt = _pack_2lvl_v3(x, twiddle, bias)
    nc = _build_2lvl_v3(repeats=_repeats)
    in_maps = [
        {"xt": xt[k], "ca": ca, "hb": hb, "bt": bt} for k in range(N_CORES)
    ]
    res = run_bass_kernel_spmd(nc, in_maps, list(range(N_CORES)))
    return _unpack_2lvl([r["out"] for r in res.results])


# --- 2lvl v4: all-bf16 matmuls, bf16 output, bias on DVE, fused h-pair
# output DMA ([128, 1024] = 2KB partition lines). Traffic: 8 MiB in +
# 8 MiB out per core vs 24 MiB for the f32-out baseline. ---


def _pack_2lvl_v4(x, twiddle, bias):
    xt, ca, hb, bt = _pack_2lvl(x, twiddle, bias, True)
    hb_bf = np.asarray(hb, np.float32).astype(ml_dtypes.bfloat16)
    return xt, ca, hb_bf, np.asarray(bt, np.float32)


def _unpack_2lvl_v4(core_outs):
    # core out: [sbt=8, S=4, m=128, (h=2)*(b=512)] -> [4096, 1024]
    parts = []
    for o in core_outs:
        arr = np.asarray(o).astype(np.float32)
        arr = arr.reshape(8, 4, 4, 32, 2, 512).transpose(0, 5, 4, 2, 1, 3)
        parts.append(arr.reshape(4096, 1024))
    return np.concatenate(parts, axis=0)


def _build_2lvl_v4(repeats: int = 1) -> bass.Bass:
    nc = bacc.Bacc()
    f32 = mybir.dt.float32
    bf16 = mybir.dt.bfloat16

    xt = nc.declare_dram_parameter("xt", [SBT_PER_CORE, P, NCHUNK, 512], bf16, isOutput=False)
    ca = nc.declare_dram_parameter("ca", [P, 8, 4, 32], bf16, isOutput=False)
    hb = nc.declare_dram_parameter("hb", [P, 4, 2, 2, P], bf16, isOutput=False)
    bt = nc.declare_dram_parameter("bt", [P, 8], f32, isOutput=False)
    out = nc.declare_dram_parameter(
        "out", [SBT_PER_CORE, 4, P, 1024], bf16, isOutput=True
    )

    with TileContext(nc) as tc:
        with (
            tc.tile_pool(name="const", bufs=1) as cpool,
            tc.tile_pool(name="xtp", bufs=2) as xpool,
            tc.tile_pool(name="zrp", bufs=2) as zrp,
            tc.tile_pool(name="outp", bufs=3) as opool,
            tc.tile_pool(name="psA", bufs=2, space="PSUM") as psA,
            tc.tile_pool(name="psO", bufs=4, space="PSUM") as psO,
        ):
            ca_sb = cpool.tile([P, 8, 4, 32], bf16)
            nc.sync.dma_start(out=ca_sb[:], in_=ca[:])
            hb_sb = cpool.tile([P, 4, 2, 2, P], bf16)
            nc.sync.dma_start(out=hb_sb[:], in_=hb[:])
            bt_sb = cpool.tile([P, 8], f32)
            nc.sync.dma_start(out=bt_sb[:], in_=bt[:])

            for _rep in range(repeats):
                for sbt in range(SBT_PER_CORE):
                    xt_sb = xpool.tile([P, NCHUNK, 512], bf16)
                    nc.sync.dma_start(out=xt_sb[:], in_=xt[sbt])
                    for S in range(4):
                        zA = psA.tile([P, 512], f32, tag="zA")
                        zB = psA.tile([P, 512], f32, tag="zB")
                        for kk in range(4):
                            nc.tensor.matmul(
                                zA[32 * kk : 32 * kk + 32, :],
                                lhsT=ca_sb[:, kk, S, :],
                                rhs=xt_sb[:, kk, :],
                                start=True, stop=True,
                                tile_position=(0, 32 * kk),
                            )
                        for kk in range(4):
                            nc.tensor.matmul(
                                zB[32 * kk : 32 * kk + 32, :],
                                lhsT=ca_sb[:, 4 + kk, S, :],
                                rhs=xt_sb[:, 4 + kk, :],
                                start=True, stop=True,
                                tile_position=(0, 32 * kk),
                            )
                        zAr = zrp.tile([P, 512], bf16, tag="zAr")
                        nc.scalar.copy(out=zAr[:], in_=zA[:])
                        zBr = zrp.tile([P, 512], bf16, tag="zBr")
                        nc.scalar.copy(out=zBr[:], in_=zB[:])
                        o_sb = opool.tile([P, 1024], bf16)
                        for h in range(2):
                            po = psO.tile([P, 512], f32)
                            nc.tensor.matmul(
                                po[:], lhsT=hb_sb[:, S, h, 0, :], rhs=zAr[:],
                                start=True, stop=False,
                            )
                            nc.tensor.matmul(
                                po[:], lhsT=hb_sb[:, S, h, 1, :], rhs=zBr[:],
                                start=False, stop=True,
                            )
                            nc.vector.tensor_scalar_add(
                                out=o_sb[:, 512 * h : 512 * h + 512],
                                in0=po[:],
                                scalar1=bt_sb[:, 2 * S + h : 2 * S + h + 1],
                            )
                        nc.sync.dma_start(out=out[sbt, S], in_=o_sb[:])
    nc.compile()
    return nc


def kernel_2lvl_v4(x, twiddle, bias, _repeats=1):
    xt, ca, hb, bt = _pack_2lvl_v4(x, twiddle, bias)
    nc = _build_2lvl_v4(repeats=_repeats)
    in_maps = [
        {"xt": xt[k], "ca": ca, "hb": hb, "bt": bt} for k in range(N_CORES)
    ]
    res = run_bass_kernel_spmd(nc, in_maps, list(range(N_CORES)))
    return _unpack_2lvl_v4([r["out"] for r in res.results])


# --- v4b: v4 with software-pipelined phase B (B(S-1) issued after A(S))
# so the PSUM->SBUF z copies never stall the PE. ---


def _build_2lvl_v4b(repeats: int = 1, tiny_out: bool = False) -> bass.Bass:
    nc = bacc.Bacc()
    f32 = mybir.dt.float32
    bf16 = mybir.dt.bfloat16
    OW = 16 if tiny_out else 1024

    xt = nc.declare_dram_parameter("xt", [SBT_PER_CORE, P, NCHUNK, 512], bf16, isOutput=False)
    ca = nc.declare_dram_parameter("ca", [P, 8, 4, 32], bf16, isOutput=False)
    hb = nc.declare_dram_parameter("hb", [P, 4, 2, 2, P], bf16, isOutput=False)
    bt = nc.declare_dram_parameter("bt", [P, 8], f32, isOutput=False)
    out = nc.declare_dram_parameter(
        "out", [SBT_PER_CORE, 4, P, OW], bf16, isOutput=True
    )

    with TileContext(nc) as tc:
        with (
            tc.tile_pool(name="const", bufs=1) as cpool,
            tc.tile_pool(name="xtp", bufs=2) as xpool,
            tc.tile_pool(name="zrp", bufs=2) as zrp,
            tc.tile_pool(name="outp", bufs=3) as opool,
            tc.tile_pool(name="psA", bufs=2, space="PSUM") as psA,
            tc.tile_pool(name="psO", bufs=4, space="PSUM") as psO,
        ):
            ca_sb = cpool.tile([P, 8, 4, 32], bf16)
            nc.sync.dma_start(out=ca_sb[:], in_=ca[:])
            hb_sb = cpool.tile([P, 4, 2, 2, P], bf16)
            nc.sync.dma_start(out=hb_sb[:], in_=hb[:])
            bt_sb = cpool.tile([P, 8], f32)
            nc.sync.dma_start(out=bt_sb[:], in_=bt[:])

            def emit_B(pend):
                sbt, S, zAr, zBr = pend
                o_sb = opool.tile([P, 1024], bf16)
                for h in range(2):
                    po = psO.tile([P, 512], f32)
                    nc.tensor.matmul(
                        po[:], lhsT=hb_sb[:, S, h, 0, :], rhs=zAr[:],
                        start=True, stop=False,
                    )
                    nc.tensor.matmul(
                        po[:], lhsT=hb_sb[:, S, h, 1, :], rhs=zBr[:],
                        start=False, stop=True,
                    )
                    nc.vector.tensor_scalar_add(
                        out=o_sb[:, 512 * h : 512 * h + 512],
                        in0=po[:],
                        scalar1=bt_sb[:, 2 * S + h : 2 * S + h + 1],
                    )
                nc.sync.dma_start(out=out[sbt, S], in_=o_sb[:, :OW])

            for _rep in range(repeats):
                pend = None
                for sbt in range(SBT_PER_CORE):
                    xt_sb = xpool.tile([P, NCHUNK, 512], bf16)
                    nc.sync.dma_start(out=xt_sb[:], in_=xt[sbt])
                    for S in range(4):
                        zA = psA.tile([P, 512], f32, tag="zA")
                        zB = psA.tile([P, 512], f32, tag="zB")
                        for kk in range(4):
                            nc.tensor.matmul(
                                zA[32 * kk : 32 * kk + 32, :],
                                lhsT=ca_sb[:, kk, S, :],
                                rhs=xt_sb[:, kk, :],
                                start=True, stop=True,
                                tile_position=(0, 32 * kk),
                            )
                        for kk in range(4):
                            nc.tensor.matmul(
                                zB[32 * kk : 32 * kk + 32, :],
                                lhsT=ca_sb[:, 4 + kk, S, :],
                                rhs=xt_sb[:, 4 + kk, :],
                                start=True, stop=True,
                                tile_position=(0, 32 * kk),
                            )
                        zAr = zrp.tile([P, 512], bf16, tag="zAr")
                        nc.scalar.copy(out=zAr[:], in_=zA[:])
                        zBr = zrp.tile([P, 512], bf16, tag="zBr")
                        nc.scalar.copy(out=zBr[:], in_=zB[:])
                        if pend is not None:
                            emit_B(pend)
                        pend = (sbt, S, zAr, zBr)
                emit_B(pend)
    nc.compile()
    return nc


def kernel_2lvl_v4b(x, twiddle, bias, _repeats=1):
    xt, ca, hb, bt = _pack_2lvl_v4(x, twiddle, bias)
    nc = _build_2lvl_v4b(repeats=_repeats)
    in_maps = [
        {"xt": xt[k], "ca": ca, "hb": hb, "bt": bt} for k in range(N_CORES)
    ]
    res = run_bass_kernel_spmd(nc, in_maps, list(range(N_CORES)))
    return _unpack_2lvl_v4([r["out"] for r in res.results])


# --- v4c: DMA ring split. Input DMAs on the SP HWDGE ring; output DMAs on
# the ACT HWDGE ring, issued right after the ACT bias-adds that produce the
# tile (same queue -> no cross-engine stall). One fused 1 MiB output DMA per
# supertile. z-copies on DVE. ---


def _unpack_2lvl_v4c(core_outs):
    # core out: [sbt=8, m=128, (S=4)*(h=2)*(b=512)] -> [4096, 1024]
    parts = []
    for o in core_outs:
        arr = np.asarray(o).astype(np.float32)
        arr = arr.reshape(8, 4, 32, 4, 2, 512).transpose(0, 5, 4, 1, 3, 2)
        parts.append(arr.reshape(4096, 1024))
    return np.concatenate(parts, axis=0)


def _build_2lvl_v4c(repeats: int = 1) -> bass.Bass:
    nc = bacc.Bacc()
    f32 = mybir.dt.float32
    bf16 = mybir.dt.bfloat16

    xt = nc.declare_dram_parameter("xt", [SBT_PER_CORE, P, NCHUNK, 512], bf16, isOutput=False)
    ca = nc.declare_dram_parameter("ca", [P, 8, 4, 32], bf16, isOutput=False)
    hb = nc.declare_dram_parameter("hb", [P, 4, 2, 2, P], bf16, isOutput=False)
    bt = nc.declare_dram_parameter("bt", [P, 8], f32, isOutput=False)
    out = nc.declare_dram_parameter("out", [SBT_PER_CORE, P, 4096], bf16, isOutput=True)

    with TileContext(nc) as tc:
        with (
            tc.tile_pool(name="const", bufs=1) as cpool,
            tc.tile_pool(name="xtp", bufs=3) as xpool,
            tc.tile_pool(name="zrp", bufs=2) as zrp,
            tc.tile_pool(name="outp", bufs=2) as opool,
            tc.tile_pool(name="psA", bufs=2, space="PSUM") as psA,
            tc.tile_pool(name="psO", bufs=4, space="PSUM") as psO,
        ):
            ca_sb = cpool.tile([P, 8, 4, 32], bf16)
            nc.sync.dma_start(out=ca_sb[:], in_=ca[:])
            hb_sb = cpool.tile([P, 4, 2, 2, P], bf16)
            nc.sync.dma_start(out=hb_sb[:], in_=hb[:])
            bt_sb = cpool.tile([P, 8], f32)
            nc.sync.dma_start(out=bt_sb[:], in_=bt[:])

            def emit_B(pend):
                sbt, S, zAr, zBr, o_sb = pend
                for h in range(2):
                    po = psO.tile([P, 512], f32)
                    nc.tensor.matmul(
                        po[:], lhsT=hb_sb[:, S, h, 0, :], rhs=zAr[:],
                        start=True, stop=False,
                    )
                    nc.tensor.matmul(
                        po[:], lhsT=hb_sb[:, S, h, 1, :], rhs=zBr[:],
                        start=False, stop=True,
                    )
                    c = 1024 * S + 512 * h
                    nc.scalar.add(
                        out=o_sb[:, c : c + 512],
                        in_=po[:],
                        add=bt_sb[:, 2 * S + h : 2 * S + h + 1],
                    )
                if S == 3:
                    nc.scalar.dma_start(out=out[sbt], in_=o_sb[:])

            for _rep in range(repeats):
                pend = None
                for sbt in range(SBT_PER_CORE):
                    xt_sb = xpool.tile([P, NCHUNK, 512], bf16)
                    nc.sync.dma_start(out=xt_sb[:], in_=xt[sbt])
                    o_sb = opool.tile([P, 4096], bf16)
                    for S in range(4):
                        zA = psA.tile([P, 512], f32, tag="zA")
                        zB = psA.tile([P, 512], f32, tag="zB")
                        for kk in range(4):
                            nc.tensor.matmul(
                                zA[32 * kk : 32 * kk + 32, :],
                                lhsT=ca_sb[:, kk, S, :],
                                rhs=xt_sb[:, kk, :],
                                start=True, stop=True,
                                tile_position=(0, 32 * kk),
                            )
                        for kk in range(4):
                            nc.tensor.matmul(
                                zB[32 * kk : 32 * kk + 32, :],
                                lhsT=ca_sb[:, 4 + kk, S, :],
                                rhs=xt_sb[:, 4 + kk, :],
                                start=True, stop=True,
                                tile_position=(0, 32 * kk),
                            )
                        zAr = zrp.tile([P, 512], bf16, tag="zAr")
                        nc.vector.tensor_copy(out=zAr[:], in_=zA[:])
                        zBr = zrp.tile([P, 512], bf16, tag="zBr")
                        nc.vector.tensor_copy(out=zBr[:], in_=zB[:])
                        if pend is not None:
                            emit_B(pend)
                        pend = (sbt, S, zAr, zBr, o_sb)
                emit_B(pend)
    nc.compile()
    return nc


def kernel_2lvl_v4c(x, twiddle, bias, _repeats=1):
    xt, ca, hb, bt = _pack_2lvl_v4(x, twiddle, bias)
    nc = _build_2lvl_v4c(repeats=_repeats)
    in_maps = [
        {"xt": xt[k], "ca": ca, "hb": hb, "bt": bt} for k in range(N_CORES)
    ]
    res = run_bass_kernel_spmd(nc, in_maps, list(range(N_CORES)))
    return _unpack_2lvl_v4c([r["out"] for r in res.results])


# --- v4d: v4c but with per-S output DMAs ([128,1024] issued right after
# each S's bias-adds on the ACT ring) — earlier issue, smaller tail. ---


def _build_2lvl_v4d(repeats: int = 1) -> bass.Bass:
    nc = bacc.Bacc()
    f32 = mybir.dt.float32
    bf16 = mybir.dt.bfloat16

    xt = nc.declare_dram_parameter("xt", [SBT_PER_CORE, P, NCHUNK, 512], bf16, isOutput=False)
    ca = nc.declare_dram_parameter("ca", [P, 8, 4, 32], bf16, isOutput=False)
    hb = nc.declare_dram_parameter("hb", [P, 4, 2, 2, P], bf16, isOutput=False)
    bt = nc.declare_dram_parameter("bt", [P, 8], f32, isOutput=False)
    out = nc.declare_dram_parameter("out", [SBT_PER_CORE, 4, P, 1024], bf16, isOutput=True)

    with TileContext(nc) as tc:
        with (
            tc.tile_pool(name="const", bufs=1) as cpool,
            tc.tile_pool(name="xtp", bufs=3) as xpool,
            tc.tile_pool(name="zrp", bufs=2) as zrp,
            tc.tile_pool(name="outp", bufs=3) as opool,
            tc.tile_pool(name="psA", bufs=2, space="PSUM") as psA,
            tc.tile_pool(name="psO", bufs=4, space="PSUM") as psO,
        ):
            ca_sb = cpool.tile([P, 8, 4, 32], bf16)
            nc.sync.dma_start(out=ca_sb[:], in_=ca[:])
            hb_sb = cpool.tile([P, 4, 2, 2, P], bf16)
            nc.sync.dma_start(out=hb_sb[:], in_=hb[:])
            bt_sb = cpool.tile([P, 8], f32)
            nc.sync.dma_start(out=bt_sb[:], in_=bt[:])

            def emit_B(pend):
                sbt, S, zAr, zBr = pend
                o_sb = opool.tile([P, 1024], bf16)
                for h in range(2):
                    po = psO.tile([P, 512], f32)
                    nc.tensor.matmul(
                        po[:], lhsT=hb_sb[:, S, h, 0, :], rhs=zAr[:],
                        start=True, stop=False,
                    )
                    nc.tensor.matmul(
                        po[:], lhsT=hb_sb[:, S, h, 1, :], rhs=zBr[:],
                        start=False, stop=True,
                    )
                    nc.scalar.add(
                        out=o_sb[:, 512 * h : 512 * h + 512],
                        in_=po[:],
                        add=bt_sb[:, 2 * S + h : 2 * S + h + 1],
                    )
                nc.scalar.dma_start(out=out[sbt, S], in_=o_sb[:])

            for _rep in range(repeats):
                pend = None
                for sbt in range(SBT_PER_CORE):
                    xt_sb = xpool.tile([P, NCHUNK, 512], bf16)
                    nc.sync.dma_start(out=xt_sb[:], in_=xt[sbt])
                    for S in range(4):
                        zA = psA.tile([P, 512], f32, tag="zA")
                        zB = psA.tile([P, 512], f32, tag="zB")
                        for kk in range(4):
                            nc.tensor.matmul(
                                zA[32 * kk : 32 * kk + 32, :],
                                lhsT=ca_sb[:, kk, S, :],
                                rhs=xt_sb[:, kk, :],
                                start=True, stop=True,
                                tile_position=(0, 32 * kk),
                            )
                        for kk in range(4):
                            nc.tensor.matmul(
                                zB[32 * kk : 32 * kk + 32, :],
                                lhsT=ca_sb[:, 4 + kk, S, :],
                                rhs=xt_sb[:, 4 + kk, :],
                                start=True, stop=True,
                                tile_position=(0, 32 * kk),
                            )
                        zAr = zrp.tile([P, 512], bf16, tag="zAr")
                        nc.vector.tensor_copy(out=zAr[:], in_=zA[:])
                        zBr = zrp.tile([P, 512], bf16, tag="zBr")
                        nc.vector.tensor_copy(out=zBr[:], in_=zB[:])
                        if pend is not None:
                            emit_B(pend)
                        pend = (sbt, S, zAr, zBr)
                emit_B(pend)
    nc.compile()
    return nc


def kernel_2lvl_v4d(x, twiddle, bias, _repeats=1):
    xt, ca, hb, bt = _pack_2lvl_v4(x, twiddle, bias)
    nc = _build_2lvl_v4d(repeats=_repeats)
    in_maps = [
        {"xt": xt[k], "ca": ca, "hb": hb, "bt": bt} for k in range(N_CORES)
    ]
    res = run_bass_kernel_spmd(nc, in_maps, list(range(N_CORES)))
    return _unpack_2lvl_v4([r["out"] for r in res.results])


def _build_dma_probe(repeats: int = 1) -> bass.Bass:
    """Pure DMA floor probe: 8 MiB in + 8 MiB out bf16, no compute."""
    nc = bacc.Bacc()
    bf16 = mybir.dt.bfloat16
    xt = nc.declare_dram_parameter("xt", [SBT_PER_CORE, P, NCHUNK, 512], bf16, isOutput=False)
    out = nc.declare_dram_parameter(
        "out", [SBT_PER_CORE, 4, P, 1024], bf16, isOutput=True
    )
    with TileContext(nc) as tc:
        with tc.tile_pool(name="xtp", bufs=3) as xpool:
            for _rep in range(repeats):
                for sbt in range(SBT_PER_CORE):
                    xt_sb = xpool.tile([P, NCHUNK, 512], bf16)
                    nc.sync.dma_start(out=xt_sb[:], in_=xt[sbt])
                    for S in range(4):
                        src = xt_sb[:, 2 * S : 2 * S + 2, :].rearrange("p a b -> p (a b)")
                        nc.sync.dma_start(out=out[sbt, S], in_=src)
    nc.compile()
    return nc





# revision 11
# speedup vs baseline: 1.8097x; 1.0226x over previous
"""Butterfly (10-stage, n=1024) as a dense composed matmul on 8 TRN2 cores.

Strategy:
  - Host: compose the 10 butterfly stage matrices into one dense W
    (1024x1024, f64 accumulate -> f32). out = x @ W^T + bias.
  - Host: pack x into PE-friendly transposed tiles so every DMA is a
    contiguous 512KB read with 4KB partition lines:
        xt[tile][c'][j][b] = x[128*tile + b, 128*j + c']
  - Device (per core, 4096 rows = 32 tiles): for each tile, 16
    accumulating matmuls (lhsT = xt chunk [c'=128, b=128] stationary,
    rhs = W^T chunk [c'=128, n=512] moving, fp32r dtype -> 1 cycle/row),
    then DVE adds bias (replicated across partitions) while moving
    PSUM->SBUF, then DMA out (contiguous 512KB).
  - Data-parallel over batch: core k handles rows [4096k, 4096(k+1)).

Variants:
  - "f32r": float32r operands (~13-bit mantissa), f32 output. ~2e-4 rel err.
  - "bf16": bf16 operands and bf16 output; halves DMA traffic. ~3e-3 rel err.
  - "dma":  DMA in/out only, no compute (perf probe).
"""

import numpy as np
import ml_dtypes

import concourse.bass as bass
import concourse.bacc as bacc
import concourse.mybir as mybir
from concourse.tile import TileContext
from concourse.bass_utils import run_bass_kernel_spmd

N_CORES = 8
BATCH = 32768
NPOS = 1024
NSTAGE = 10
P = 128
NCHUNK = NPOS // P  # 8
TILES_PER_CORE = BATCH // N_CORES // P  # 32

VARIANT = "f32r"


def _compose_w(twiddle: np.ndarray) -> np.ndarray:
    """Compose the butterfly stages into M_id[c, n] = W[n, c] (= W^T).

    Applies the reference butterfly to the identity matrix in float64.
    Row c of the result is B @ e_c, i.e. column c of the composed W.
    """
    tw = np.asarray(twiddle, dtype=np.float64)  # (1, 10, 512, 2, 2)
    n = NPOS
    out = np.eye(n, dtype=np.float64).reshape(n, 1, n)
    for idx in range(NSTAGE):
        stride = 1 << idx
        nb = n // (2 * stride)
        t = tw[:, idx].reshape(1, nb, stride, 2, 2).transpose(0, 1, 3, 4, 2)
        o = out.reshape(n, 1, nb, 1, 2, stride)
        out = (t * o).sum(axis=4).reshape(n, 1, n)
    return out.reshape(n, n)  # [c, n]


def _build_nc(variant: str = VARIANT, repeats: int = 1) -> bass.Bass:
    nc = bacc.Bacc()
    f32 = mybir.dt.float32

    if variant == "bf16":
        in_dt = mybir.dt.bfloat16
        out_dt = mybir.dt.bfloat16
    else:
        in_dt = mybir.dt.float32r
        out_dt = f32

    xt = nc.declare_dram_parameter(
        "xt", [TILES_PER_CORE, P, NCHUNK, P], in_dt, isOutput=False
    )
    w = nc.declare_dram_parameter("w", [P, NCHUNK, NPOS], in_dt, isOutput=False)
    bias = nc.declare_dram_parameter("bias", [P, NPOS], f32, isOutput=False)
    out = nc.declare_dram_parameter(
        "out", [TILES_PER_CORE, P, NPOS], out_dt, isOutput=True
    )

    with TileContext(nc) as tc:
        with (
            tc.tile_pool(name="const", bufs=1) as cpool,
            tc.tile_pool(name="xtp", bufs=3) as xpool,
            tc.tile_pool(name="outp", bufs=3) as opool,
            tc.tile_pool(name="ps", bufs=4, space="PSUM") as pspool,
        ):
            w_sb = cpool.tile([P, NCHUNK, NPOS], in_dt)
            nc.sync.dma_start(out=w_sb[:], in_=w[:])
            b_sb = cpool.tile([P, NPOS], f32)
            nc.sync.dma_start(out=b_sb[:], in_=bias[:])

            for _rep in range(repeats):
                for t in range(TILES_PER_CORE):
                    xt_sb = xpool.tile([P, NCHUNK, P], in_dt)
                    nc.sync.dma_start(out=xt_sb[:], in_=xt[t])
                    o_sb = opool.tile([P, NPOS], out_dt)
                    if variant != "dma":
                        for nh in range(2):
                            ns = nh * 512
                            ps = pspool.tile([P, 512], f32)
                            for j in range(NCHUNK):
                                nc.tensor.matmul(
                                    ps[:],
                                    lhsT=xt_sb[:, j, :],
                                    rhs=w_sb[:, j, ns : ns + 512],
                                    start=(j == 0),
                                    stop=(j == NCHUNK - 1),
                                )
                            nc.vector.tensor_add(
                                out=o_sb[:, ns : ns + 512],
                                in0=ps[:],
                                in1=b_sb[:, ns : ns + 512],
                            )
                    if variant == "dma":
                        src = xt_sb[:].rearrange("p a b -> p (a b)").bitcast(out_dt)
                        nc.sync.dma_start(out=out[t], in_=src)
                    else:
                        nc.sync.dma_start(out=out[t], in_=o_sb[:])
    nc.compile()
    return nc


def _pack_inputs(x, twiddle, bias, variant: str = VARIANT):
    x = np.asarray(x, dtype=np.float32)
    bias = np.asarray(bias, dtype=np.float32)

    m_id = _compose_w(twiddle).astype(np.float32)  # [c, n] = W^T
    w_packed = np.ascontiguousarray(
        m_id.reshape(NCHUNK, P, NPOS).transpose(1, 0, 2)
    )  # [c', j, n]
    bias_rep = np.ascontiguousarray(np.broadcast_to(bias, (P, NPOS)))

    # [ntile, c', j, b] with ntile = 256 global tiles of 128 rows
    xt_all = np.ascontiguousarray(
        x.reshape(BATCH // P, P, NCHUNK, P).transpose(0, 3, 2, 1)
    )
    if variant == "bf16":
        xt_all = xt_all.astype(ml_dtypes.bfloat16)
        w_packed = w_packed.astype(ml_dtypes.bfloat16)
    return xt_all, w_packed, bias_rep


def kernel(x, twiddle, bias, _variant: str = "2lvl_v4c", _repeats: int = 1):
    """Harness entry point: full inputs in, full output out.

    Default path (2lvl_v4c): two-level butterfly factorization, all-bf16
    matmuls (stages 0-6 as col-tiled block-diagonal matmuls exploiting PE
    sub-array concurrency, stages 7-9 as K=128 matmuls in band-mixed z
    space), bias added on ACT, bf16 output. Input DMAs ride the SP HWDGE
    ring, output DMAs the ACT ring (issued right after the bias-adds that
    produce them) so transfers overlap compute. Measured ~41.6us/pass on
    8 cores (DMA floor ~39.7us for 8MiB in + 8MiB out bf16 at ~423GB/s),
    max rel err ~6.4e-3.
    Fallback _variant="2lvl": older f32-out pipeline, ~79us/pass.
    Fallback _variant="f32r": dense composed-W f32r kernel,
    ~100-150us/pass, max rel err ~2e-4.
    """
    if _variant == "2lvl_v4c":
        return kernel_2lvl_v4c(x, twiddle, bias, _repeats=_repeats)
    if _variant == "2lvl":
        return kernel_2lvl(x, twiddle, bias, out_bf16=False, _repeats=_repeats)
    xt_all, w_packed, bias_rep = _pack_inputs(x, twiddle, bias, _variant)

    nc = _build_nc(variant=_variant, repeats=_repeats)
    in_maps = [
        {
            "xt": xt_all[k * TILES_PER_CORE : (k + 1) * TILES_PER_CORE],
            "w": w_packed,
            "bias": bias_rep,
        }
        for k in range(N_CORES)
    ]
    res = run_bass_kernel_spmd(nc, in_maps, list(range(N_CORES)))

    out = np.concatenate(
        [np.asarray(r["out"]).reshape(-1, NPOS) for r in res.results], axis=0
    ).astype(np.float32)
    return out


# ---------------------------------------------------------------------------
# Two-level factorization: stages 0-6 (block-diag, col-tiled bf16 matmuls)
# then stages 7-9 (16 accumulating f32r matmuls), position-major orientation.
# Output is produced transposed ([pos, batch]); host re-transposes.
# ---------------------------------------------------------------------------

SBT_PER_CORE = 8  # super-tiles of 512 batch rows per core


def _apply_stages(tw, v, stages):
    b, n = v.shape
    out = v.reshape(b, 1, n)
    tw = np.asarray(tw, dtype=np.float64)
    for idx in stages:
        stride = 1 << idx
        nb = n // (2 * stride)
        t = tw[:, idx].reshape(1, nb, stride, 2, 2).transpose(0, 1, 3, 4, 2)
        o = out.reshape(b, 1, nb, 1, 2, stride)
        out = (t * o).sum(axis=4).reshape(b, 1, n)
    return out.reshape(b, n)


def _pack_2lvl(x, twiddle, bias, out_bf16: bool):
    x = np.asarray(x, dtype=np.float32)
    bias = np.asarray(bias, dtype=np.float64)
    n = NPOS
    I = np.eye(n)
    C_full = _apply_stages(twiddle, I, range(0, 7)).T  # [p, c]
    H = _apply_stages(twiddle, I, range(7, 10)).T      # [p', p]

    ca = np.empty((128, 8, 4, 32), np.float32)  # [c, k, S, m]
    for k in range(8):
        blk = C_full[128 * k : 128 * k + 128, 128 * k : 128 * k + 128]
        for S in range(4):
            ca[:, k, S, :] = blk[32 * S : 32 * S + 32, :].T
    ca = ca.astype(ml_dtypes.bfloat16)

    hb = np.empty((128, 4, 2, 2, 128), np.float32)  # [q, S, h, z, m]
    bt = np.empty((128, 8), np.float32)             # [q, 2S+h]
    for S in range(4):
        for h in range(2):
            rows_m = np.array(
                [128 * (4 * h + j) + 32 * S + s2 for j in range(4) for s2 in range(32)]
            )
            for z in range(2):
                cols_q = np.array(
                    [128 * (4 * z + k) + 32 * S + s for k in range(4) for s in range(32)]
                )
                hb[:, S, h, z, :] = H[np.ix_(rows_m, cols_q)].T
            bt[:, 2 * S + h] = bias[rows_m]
    bt = bt.astype(np.float32)

    # xt: [ncores, sbt, c', j, b] bf16
    xt = np.ascontiguousarray(
        x.reshape(N_CORES, SBT_PER_CORE, 512, NCHUNK, P).transpose(0, 1, 4, 3, 2)
    ).astype(ml_dtypes.bfloat16)
    return xt, ca, hb, bt


def _unpack_2lvl(core_outs):
    # core out: [sbt=8, S=4, h=2, m=128, b=512] -> [4096, 1024]
    parts = []
    for o in core_outs:
        arr = np.asarray(o).astype(np.float32)
        arr = arr.reshape(8, 4, 2, 4, 32, 512).transpose(0, 5, 2, 3, 1, 4)
        parts.append(arr.reshape(4096, 1024))
    return np.concatenate(parts, axis=0)


def _build_2lvl(out_bf16: bool, repeats: int = 1, xtp_bufs: int = 3, zrp_bufs: int = 3, outp_bufs: int = 6) -> bass.Bass:
    nc = bacc.Bacc()
    f32 = mybir.dt.float32
    f32r = mybir.dt.float32r
    bf16 = mybir.dt.bfloat16
    out_dt = bf16 if out_bf16 else f32

    xt = nc.declare_dram_parameter("xt", [SBT_PER_CORE, P, NCHUNK, 512], bf16, isOutput=False)
    ca = nc.declare_dram_parameter("ca", [P, 8, 4, 32], bf16, isOutput=False)
    hb = nc.declare_dram_parameter("hb", [P, 4, 2, 2, P], f32r, isOutput=False)
    bt = nc.declare_dram_parameter("bt", [P, 8], f32, isOutput=False)
    out = nc.declare_dram_parameter(
        "out", [SBT_PER_CORE, 4, 2, P, 512], out_dt, isOutput=True
    )

    with TileContext(nc) as tc:
        with (
            tc.tile_pool(name="const", bufs=1) as cpool,
            tc.tile_pool(name="xtp", bufs=xtp_bufs) as xpool,
            tc.tile_pool(name="zrp", bufs=zrp_bufs) as zrp,
            tc.tile_pool(name="outp", bufs=outp_bufs) as opool,
            tc.tile_pool(name="psA", bufs=2, space="PSUM") as psA,
            tc.tile_pool(name="psO", bufs=4, space="PSUM") as psO,
        ):
            ca_sb = cpool.tile([P, 8, 4, 32], bf16)
            nc.sync.dma_start(out=ca_sb[:], in_=ca[:])
            hb_sb = cpool.tile([P, 4, 2, 2, P], f32r)
            nc.sync.dma_start(out=hb_sb[:], in_=hb[:])
            bt_sb = cpool.tile([P, 8], f32)
            nc.sync.dma_start(out=bt_sb[:], in_=bt[:])

            for _rep in range(repeats):
                for sbt in range(SBT_PER_CORE):
                    xt_sb = xpool.tile([P, NCHUNK, 512], bf16)
                    nc.sync.dma_start(out=xt_sb[:], in_=xt[sbt])
                    for S in range(4):
                        zA = psA.tile([P, 512], f32, tag="zA")
                        zB = psA.tile([P, 512], f32, tag="zB")
                        for kk in range(4):
                            nc.tensor.matmul(
                                zA[32 * kk : 32 * kk + 32, :],
                                lhsT=ca_sb[:, kk, S, :],
                                rhs=xt_sb[:, kk, :],
                                start=True, stop=True,
                                tile_position=(0, 32 * kk),
                            )
                        for kk in range(4):
                            nc.tensor.matmul(
                                zB[32 * kk : 32 * kk + 32, :],
                                lhsT=ca_sb[:, 4 + kk, S, :],
                                rhs=xt_sb[:, 4 + kk, :],
                                start=True, stop=True,
                                tile_position=(0, 32 * kk),
                            )
                        zAr = zrp.tile([P, 512], f32r, tag="zAr")
                        nc.scalar.copy(out=zAr[:], in_=zA[:])
                        zBr = zrp.tile([P, 512], f32r, tag="zBr")
                        nc.scalar.copy(out=zBr[:], in_=zB[:])
                        for h in range(2):
                            po = psO.tile([P, 512], f32)
                            nc.tensor.matmul(
                                po[:], lhsT=hb_sb[:, S, h, 0, :], rhs=zAr[:],
                                start=True, stop=False,
                            )
                            nc.tensor.matmul(
                                po[:], lhsT=hb_sb[:, S, h, 1, :], rhs=zBr[:],
                                start=False, stop=True,
                            )
                            o_sb = opool.tile([P, 512], out_dt)
                            nc.vector.tensor_scalar_add(
                                out=o_sb[:], in0=po[:],
                                scalar1=bt_sb[:, 2 * S + h : 2 * S + h + 1],
                            )
                            nc.sync.dma_start(out=out[sbt, S, h], in_=o_sb[:])
    nc.compile()
    return nc


def kernel_2lvl(x, twiddle, bias, out_bf16=False, _repeats=1):
    xt, ca, hb, bt = _pack_2lvl(x, twiddle, bias, out_bf16)
    nc = _build_2lvl(out_bf16, repeats=_repeats)
    in_maps = [
        {"xt": xt[k], "ca": ca, "hb": hb, "bt": bt} for k in range(N_CORES)
    ]
    res = run_bass_kernel_spmd(nc, in_maps, list(range(N_CORES)))
    return _unpack_2lvl([r["out"] for r in res.results])


# --- 2lvl v2: z-copies as bf16 on DVE, phase B bf16, bias via K=1 matmul ---

def _pack_2lvl_v2(x, twiddle, bias):
    xt, ca, hb, bt = _pack_2lvl(x, twiddle, bias, True)
    hb_bf = np.asarray(hb, np.float32).astype(ml_dtypes.bfloat16)
    # bias as [1, 8, 128]: bt2[0, 2S+h, m]
    bt2 = np.ascontiguousarray(np.asarray(bt, np.float32).T.reshape(1, 8, 128)).astype(
        ml_dtypes.bfloat16
    )
    return xt, ca, hb_bf, bt2


def _build_2lvl_v2(repeats: int = 1) -> bass.Bass:
    nc = bacc.Bacc()
    f32 = mybir.dt.float32
    bf16 = mybir.dt.bfloat16

    xt = nc.declare_dram_parameter("xt", [SBT_PER_CORE, P, NCHUNK, 512], bf16, isOutput=False)
    ca = nc.declare_dram_parameter("ca", [P, 8, 4, 32], bf16, isOutput=False)
    hb = nc.declare_dram_parameter("hb", [P, 4, 2, 2, P], bf16, isOutput=False)
    bt = nc.declare_dram_parameter("bt", [1, 8, P], bf16, isOutput=False)
    out = nc.declare_dram_parameter(
        "out", [SBT_PER_CORE, 4, 2, P, 512], bf16, isOutput=True
    )

    with TileContext(nc) as tc:
        with (
            tc.tile_pool(name="const", bufs=1) as cpool,
            tc.tile_pool(name="xtp", bufs=2) as xpool,
            tc.tile_pool(name="zrp", bufs=2) as zrp,
            tc.tile_pool(name="outp", bufs=4) as opool,
            tc.tile_pool(name="psA", bufs=2, space="PSUM") as psA,
            tc.tile_pool(name="psO", bufs=4, space="PSUM") as psO,
        ):
            ca_sb = cpool.tile([P, 8, 4, 32], bf16)
            nc.sync.dma_start(out=ca_sb[:], in_=ca[:])
            hb_sb = cpool.tile([P, 4, 2, 2, P], bf16)
            nc.sync.dma_start(out=hb_sb[:], in_=hb[:])
            bt_sb = cpool.tile([1, 8, P], bf16)
            nc.sync.dma_start(out=bt_sb[:], in_=bt[:])
            ones_sb = cpool.tile([1, 512], bf16)
            nc.vector.memset(ones_sb[:], 1.0)

            for _rep in range(repeats):
                for sbt in range(SBT_PER_CORE):
                    xt_sb = xpool.tile([P, NCHUNK, 512], bf16)
                    nc.sync.dma_start(out=xt_sb[:], in_=xt[sbt])
                    for S in range(4):
                        zA = psA.tile([P, 512], f32, tag="zA")
                        zB = psA.tile([P, 512], f32, tag="zB")
                        for kk in range(4):
                            nc.tensor.matmul(
                                zA[32 * kk : 32 * kk + 32, :],
                                lhsT=ca_sb[:, kk, S, :],
                                rhs=xt_sb[:, kk, :],
                                start=True, stop=True,
                                tile_position=(0, 32 * kk),
                            )
                        for kk in range(4):
                            nc.tensor.matmul(
                                zB[32 * kk : 32 * kk + 32, :],
                                lhsT=ca_sb[:, 4 + kk, S, :],
                                rhs=xt_sb[:, 4 + kk, :],
                                start=True, stop=True,
                                tile_position=(0, 32 * kk),
                            )
                        zAr = zrp.tile([P, 512], bf16, tag="zAr")
                        nc.vector.tensor_copy(out=zAr[:], in_=zA[:])
                        zBr = zrp.tile([P, 512], bf16, tag="zBr")
                        nc.vector.tensor_copy(out=zBr[:], in_=zB[:])
                        for h in range(2):
                            po = psO.tile([P, 512], f32)
                            nc.tensor.matmul(
                                po[:], lhsT=bt_sb[:, 2 * S + h, :], rhs=ones_sb[:],
                                start=True, stop=False,
                            )
                            nc.tensor.matmul(
                                po[:], lhsT=hb_sb[:, S, h, 0, :], rhs=zAr[:],
                                start=False, stop=False,
                            )
                            nc.tensor.matmul(
                                po[:], lhsT=hb_sb[:, S, h, 1, :], rhs=zBr[:],
                                start=False, stop=True,
                            )
                            o_sb = opool.tile([P, 512], bf16)
                            nc.vector.tensor_copy(out=o_sb[:], in_=po[:])
                            nc.sync.dma_start(out=out[sbt, S, h], in_=o_sb[:])
    nc.compile()
    return nc


def kernel_2lvl_v2(x, twiddle, bias, _repeats=1):
    xt, ca, hb, bt = _pack_2lvl_v2(x, twiddle, bias)
    nc = _build_2lvl_v2(repeats=_repeats)
    in_maps = [
        {"xt": xt[k], "ca": ca, "hb": hb, "bt": bt} for k in range(N_CORES)
    ]
    res = run_bass_kernel_spmd(nc, in_maps, list(range(N_CORES)))
    return _unpack_2lvl([r["out"] for r in res.results])


# --- 2lvl v3: bf16 out, bias as K=1 matmul on PE, out-copies split ACT/DVE ---

def _pack_2lvl_v3(x, twiddle, bias):
    xt, ca, hb, bt = _pack_2lvl(x, twiddle, bias, True)
    # bias as [1, 8, 128] bf16 for the K=1 matmul: bt2[0, 2S+h, m]
    bt2 = np.ascontiguousarray(np.asarray(bt, np.float32).T.reshape(1, 8, 128)).astype(
        ml_dtypes.bfloat16
    )
    return xt, ca, hb, bt2


def _build_2lvl_v3(repeats: int = 1) -> bass.Bass:
    nc = bacc.Bacc()
    f32 = mybir.dt.float32
    f32r = mybir.dt.float32r
    bf16 = mybir.dt.bfloat16

    xt = nc.declare_dram_parameter("xt", [SBT_PER_CORE, P, NCHUNK, 512], bf16, isOutput=False)
    ca = nc.declare_dram_parameter("ca", [P, 8, 4, 32], bf16, isOutput=False)
    hb = nc.declare_dram_parameter("hb", [P, 4, 2, 2, P], f32r, isOutput=False)
    bt = nc.declare_dram_parameter("bt", [1, 8, P], bf16, isOutput=False)
    out = nc.declare_dram_parameter(
        "out", [SBT_PER_CORE, 4, 2, P, 512], bf16, isOutput=True
    )

    with TileContext(nc) as tc:
        with (
            tc.tile_pool(name="const", bufs=1) as cpool,
            tc.tile_pool(name="xtp", bufs=2) as xpool,
            tc.tile_pool(name="zrp", bufs=2) as zrp,
            tc.tile_pool(name="outp", bufs=4) as opool,
            tc.tile_pool(name="psA", bufs=2, space="PSUM") as psA,
            tc.tile_pool(name="psO", bufs=4, space="PSUM") as psO,
        ):
            ca_sb = cpool.tile([P, 8, 4, 32], bf16)
            nc.sync.dma_start(out=ca_sb[:], in_=ca[:])
            hb_sb = cpool.tile([P, 4, 2, 2, P], f32r)
            nc.sync.dma_start(out=hb_sb[:], in_=hb[:])
            bt_sb = cpool.tile([1, 8, P], bf16)
            nc.sync.dma_start(out=bt_sb[:], in_=bt[:])
            ones_sb = cpool.tile([1, 512], bf16)
            nc.vector.memset(ones_sb[:], 1.0)

            for _rep in range(repeats):
                for sbt in range(SBT_PER_CORE):
                    xt_sb = xpool.tile([P, NCHUNK, 512], bf16)
                    nc.sync.dma_start(out=xt_sb[:], in_=xt[sbt])
                    for S in range(4):
                        zA = psA.tile([P, 512], f32, tag="zA")
                        zB = psA.tile([P, 512], f32, tag="zB")
                        for kk in range(4):
                            nc.tensor.matmul(
                                zA[32 * kk : 32 * kk + 32, :],
                                lhsT=ca_sb[:, kk, S, :],
                                rhs=xt_sb[:, kk, :],
                                start=True, stop=True,
                                tile_position=(0, 32 * kk),
                            )
                        for kk in range(4):
                            nc.tensor.matmul(
                                zB[32 * kk : 32 * kk + 32, :],
                                lhsT=ca_sb[:, 4 + kk, S, :],
                                rhs=xt_sb[:, 4 + kk, :],
                                start=True, stop=True,
                                tile_position=(0, 32 * kk),
                            )
                        zAr = zrp.tile([P, 512], f32r, tag="zAr")
                        nc.scalar.copy(out=zAr[:], in_=zA[:])
                        zBr = zrp.tile([P, 512], f32r, tag="zBr")
                        nc.scalar.copy(out=zBr[:], in_=zB[:])
                        for h in range(2):
                            po = psO.tile([P, 512], f32)
                            nc.tensor.matmul(
                                po[:], lhsT=bt_sb[:, 2 * S + h, :], rhs=ones_sb[:],
                                start=True, stop=False,
                            )
                            nc.tensor.matmul(
                                po[:], lhsT=hb_sb[:, S, h, 0, :], rhs=zAr[:],
                                start=False, stop=False,
                            )
                            nc.tensor.matmul(
                                po[:], lhsT=hb_sb[:, S, h, 1, :], rhs=zBr[:],
                                start=False, stop=True,
                            )
                            o_sb = opool.tile([P, 512], bf16)
                            if (2 * S + h) % 2 == 0:
                                nc.scalar.copy(out=o_sb[:], in_=po[:])
                            else:
                                nc.vector.tensor_copy(out=o_sb[:], in_=po[:])
                            nc.sync.dma_start(out=out[sbt, S, h], in_=o_sb[:])
    nc.compile()
    return nc


def kernel_2lvl_v3(x, twiddle, bias, _repeats=1):
    xt, ca, hb, bt = _pack_2lvl_v3(x, twiddle, bias)
    nc = _build_2lvl_v3(repeats=_repeats)
    in_maps = [
        {"xt": xt[k], "ca": ca, "hb": hb, "bt": bt} for k in range(N_CORES)
    ]
    res = run_bass_kernel_spmd(nc, in_maps, list(range(N_CORES)))
    return _unpack_2lvl([r["out"] for r in res.results])


# --- 2lvl v4: all-bf16 matmuls, bf16 output, bias on DVE, fused h-pair
# output DMA ([128, 1024] = 2KB partition lines). Traffic: 8 MiB in +
# 8 MiB out per core vs 24 MiB for the f32-out baseline. ---


def _pack_2lvl_v4(x, twiddle, bias):
    xt, ca, hb, bt = _pack_2lvl(x, twiddle, bias, True)
    hb_bf = np.asarray(hb, np.float32).astype(ml_dtypes.bfloat16)
    return xt, ca, hb_bf, np.asarray(bt, np.float32)


def _unpack_2lvl_v4(core_outs):
    # core out: [sbt=8, S=4, m=128, (h=2)*(b=512)] -> [4096, 1024]
    parts = []
    for o in core_outs:
        arr = np.asarray(o).astype(np.float32)
        arr = arr.reshape(8, 4, 4, 32, 2, 512).transpose(0, 5, 4, 2, 1, 3)
        parts.append(arr.reshape(4096, 1024))
    return np.concatenate(parts, axis=0)


def _build_2lvl_v4(repeats: int = 1) -> bass.Bass:
    nc = bacc.Bacc()
    f32 = mybir.dt.float32
    bf16 = mybir.dt.bfloat16

    xt = nc.declare_dram_parameter("xt", [SBT_PER_CORE, P, NCHUNK, 512], bf16, isOutput=False)
    ca = nc.declare_dram_parameter("ca", [P, 8, 4, 32], bf16, isOutput=False)
    hb = nc.declare_dram_parameter("hb", [P, 4, 2, 2, P], bf16, isOutput=False)
    bt = nc.declare_dram_parameter("bt", [P, 8], f32, isOutput=False)
    out = nc.declare_dram_parameter(
        "out", [SBT_PER_CORE, 4, P, 1024], bf16, isOutput=True
    )

    with TileContext(nc) as tc:
        with (
            tc.tile_pool(name="const", bufs=1) as cpool,
            tc.tile_pool(name="xtp", bufs=2) as xpool,
            tc.tile_pool(name="zrp", bufs=2) as zrp,
            tc.tile_pool(name="outp", bufs=3) as opool,
            tc.tile_pool(name="psA", bufs=2, space="PSUM") as psA,
            tc.tile_pool(name="psO", bufs=4, space="PSUM") as psO,
        ):
            ca_sb = cpool.tile([P, 8, 4, 32], bf16)
            nc.sync.dma_start(out=ca_sb[:], in_=ca[:])
            hb_sb = cpool.tile([P, 4, 2, 2, P], bf16)
            nc.sync.dma_start(out=hb_sb[:], in_=hb[:])
            bt_sb = cpool.tile([P, 8], f32)
            nc.sync.dma_start(out=bt_sb[:], in_=bt[:])

            for _rep in range(repeats):
                for sbt in range(SBT_PER_CORE):
                    xt_sb = xpool.tile([P, NCHUNK, 512], bf16)
                    nc.sync.dma_start(out=xt_sb[:], in_=xt[sbt])
                    for S in range(4):
                        zA = psA.tile([P, 512], f32, tag="zA")
                        zB = psA.tile([P, 512], f32, tag="zB")
                        for kk in range(4):
                            nc.tensor.matmul(
                                zA[32 * kk : 32 * kk + 32, :],
                                lhsT=ca_sb[:, kk, S, :],
                                rhs=xt_sb[:, kk, :],
                                start=True, stop=True,
                                tile_position=(0, 32 * kk),
                            )
                        for kk in range(4):
                            nc.tensor.matmul(
                                zB[32 * kk : 32 * kk + 32, :],
                                lhsT=ca_sb[:, 4 + kk, S, :],
                                rhs=xt_sb[:, 4 + kk, :],
                                start=True, stop=True,
                                tile_position=(0, 32 * kk),
                            )
                        zAr = zrp.tile([P, 512], bf16, tag="zAr")
                        nc.scalar.copy(out=zAr[:], in_=zA[:])
                        zBr = zrp.tile([P, 512], bf16, tag="zBr")
                        nc.scalar.copy(out=zBr[:], in_=zB[:])
                        o_sb = opool.tile([P, 1024], bf16)
                        for h in range(2):
                            po = psO.tile([P, 512], f32)
                            nc.tensor.matmul(
                                po[:], lhsT=hb_sb[:, S, h, 0, :], rhs=zAr[:],
                                start=True, stop=False,
                            )
                            nc.tensor.matmul(
                                po[:], lhsT=hb_sb[:, S, h, 1, :], rhs=zBr[:],
                                start=False, stop=True,
                            )
                            nc.vector.tensor_scalar_add(
                                out=o_sb[:, 512 * h : 512 * h + 512],
                                in0=po[:],
                                scalar1=bt_sb[:, 2 * S + h : 2 * S + h + 1],
                            )
                        nc.sync.dma_start(out=out[sbt, S], in_=o_sb[:])
    nc.compile()
    return nc


def kernel_2lvl_v4(x, twiddle, bias, _repeats=1):
    xt, ca, hb, bt = _pack_2lvl_v4(x, twiddle, bias)
    nc = _build_2lvl_v4(repeats=_repeats)
    in_maps = [
        {"xt": xt[k], "ca": ca, "hb": hb, "bt": bt} for k in range(N_CORES)
    ]
    res = run_bass_kernel_spmd(nc, in_maps, list(range(N_CORES)))
    return _unpack_2lvl_v4([r["out"] for r in res.results])


# --- v4b: v4 with software-pipelined phase B (B(S-1) issued after A(S))
# so the PSUM->SBUF z copies never stall the PE. ---


def _build_2lvl_v4b(repeats: int = 1, tiny_out: bool = False) -> bass.Bass:
    nc = bacc.Bacc()
    f32 = mybir.dt.float32
    bf16 = mybir.dt.bfloat16
    OW = 16 if tiny_out else 1024

    xt = nc.declare_dram_parameter("xt", [SBT_PER_CORE, P, NCHUNK, 512], bf16, isOutput=False)
    ca = nc.declare_dram_parameter("ca", [P, 8, 4, 32], bf16, isOutput=False)
    hb = nc.declare_dram_parameter("hb", [P, 4, 2, 2, P], bf16, isOutput=False)
    bt = nc.declare_dram_parameter("bt", [P, 8], f32, isOutput=False)
    out = nc.declare_dram_parameter(
        "out", [SBT_PER_CORE, 4, P, OW], bf16, isOutput=True
    )

    with TileContext(nc) as tc:
        with (
            tc.tile_pool(name="const", bufs=1) as cpool,
            tc.tile_pool(name="xtp", bufs=2) as xpool,
            tc.tile_pool(name="zrp", bufs=2) as zrp,
            tc.tile_pool(name="outp", bufs=3) as opool,
            tc.tile_pool(name="psA", bufs=2, space="PSUM") as psA,
            tc.tile_pool(name="psO", bufs=4, space="PSUM") as psO,
        ):
            ca_sb = cpool.tile([P, 8, 4, 32], bf16)
            nc.sync.dma_start(out=ca_sb[:], in_=ca[:])
            hb_sb = cpool.tile([P, 4, 2, 2, P], bf16)
            nc.sync.dma_start(out=hb_sb[:], in_=hb[:])
            bt_sb = cpool.tile([P, 8], f32)
            nc.sync.dma_start(out=bt_sb[:], in_=bt[:])

            def emit_B(pend):
                sbt, S, zAr, zBr = pend
                o_sb = opool.tile([P, 1024], bf16)
                for h in range(2):
                    po = psO.tile([P, 512], f32)
                    nc.tensor.matmul(
                        po[:], lhsT=hb_sb[:, S, h, 0, :], rhs=zAr[:],
                        start=True, stop=False,
                    )
                    nc.tensor.matmul(
                        po[:], lhsT=hb_sb[:, S, h, 1, :], rhs=zBr[:],
                        start=False, stop=True,
                    )
                    nc.vector.tensor_scalar_add(
                        out=o_sb[:, 512 * h : 512 * h + 512],
                        in0=po[:],
                        scalar1=bt_sb[:, 2 * S + h : 2 * S + h + 1],
                    )
                nc.sync.dma_start(out=out[sbt, S], in_=o_sb[:, :OW])

            for _rep in range(repeats):
                pend = None
                for sbt in range(SBT_PER_CORE):
                    xt_sb = xpool.tile([P, NCHUNK, 512], bf16)
                    nc.sync.dma_start(out=xt_sb[:], in_=xt[sbt])
                    for S in range(4):
                        zA = psA.tile([P, 512], f32, tag="zA")
                        zB = psA.tile([P, 512], f32, tag="zB")
                        for kk in range(4):
                            nc.tensor.matmul(
                                zA[32 * kk : 32 * kk + 32, :],
                                lhsT=ca_sb[:, kk, S, :],
                                rhs=xt_sb[:, kk, :],
                                start=True, stop=True,
                                tile_position=(0, 32 * kk),
                            )
                        for kk in range(4):
                            nc.tensor.matmul(
                                zB[32 * kk : 32 * kk + 32, :],
                                lhsT=ca_sb[:, 4 + kk, S, :],
                                rhs=xt_sb[:, 4 + kk, :],
                                start=True, stop=True,
                                tile_position=(0, 32 * kk),
                            )
                        zAr = zrp.tile([P, 512], bf16, tag="zAr")
                        nc.scalar.copy(out=zAr[:], in_=zA[:])
                        zBr = zrp.tile([P, 512], bf16, tag="zBr")
                        nc.scalar.copy(out=zBr[:], in_=zB[:])
                        if pend is not None:
                            emit_B(pend)
                        pend = (sbt, S, zAr, zBr)
                emit_B(pend)
    nc.compile()
    return nc


def kernel_2lvl_v4b(x, twiddle, bias, _repeats=1):
    xt, ca, hb, bt = _pack_2lvl_v4(x, twiddle, bias)
    nc = _build_2lvl_v4b(repeats=_repeats)
    in_maps = [
        {"xt": xt[k], "ca": ca, "hb": hb, "bt": bt} for k in range(N_CORES)
    ]
    res = run_bass_kernel_spmd(nc, in_maps, list(range(N_CORES)))
    return _unpack_2lvl_v4([r["out"] for r in res.results])


# --- v4c: DMA ring split. Input DMAs on the SP HWDGE ring; output DMAs on
# the ACT HWDGE ring, issued right after the ACT bias-adds that produce the
# tile (same queue -> no cross-engine stall). One fused 1 MiB output DMA per
# supertile. z-copies on DVE. ---


def _unpack_2lvl_v4c(core_outs):
    # core out: [sbt=8, m=128, (S=4)*(h=2)*(b=512)] -> [4096, 1024]
    parts = []
    for o in core_outs:
        arr = np.asarray(o).astype(np.float32)
        arr = arr.reshape(8, 4, 32, 4, 2, 512).transpose(0, 5, 4, 1, 3, 2)
        parts.append(arr.reshape(4096, 1024))
    return np.concatenate(parts, axis=0)


def _build_2lvl_v4c(repeats: int = 1, xtp_bufs: int = 3, zrp_bufs: int = 2, outp_bufs: int = 2) -> bass.Bass:
    nc = bacc.Bacc()
    f32 = mybir.dt.float32
    bf16 = mybir.dt.bfloat16

    xt = nc.declare_dram_parameter("xt", [SBT_PER_CORE, P, NCHUNK, 512], bf16, isOutput=False)
    ca = nc.declare_dram_parameter("ca", [P, 8, 4, 32], bf16, isOutput=False)
    hb = nc.declare_dram_parameter("hb", [P, 4, 2, 2, P], bf16, isOutput=False)
    bt = nc.declare_dram_parameter("bt", [P, 8], f32, isOutput=False)
    out = nc.declare_dram_parameter("out", [SBT_PER_CORE, P, 4096], bf16, isOutput=True)

    with TileContext(nc) as tc:
        with (
            tc.tile_pool(name="const", bufs=1) as cpool,
            tc.tile_pool(name="xtp", bufs=xtp_bufs) as xpool,
            tc.tile_pool(name="zrp", bufs=zrp_bufs) as zrp,
            tc.tile_pool(name="outp", bufs=outp_bufs) as opool,
            tc.tile_pool(name="psA", bufs=2, space="PSUM") as psA,
            tc.tile_pool(name="psO", bufs=4, space="PSUM") as psO,
        ):
            ca_sb = cpool.tile([P, 8, 4, 32], bf16)
            nc.sync.dma_start(out=ca_sb[:], in_=ca[:])
            hb_sb = cpool.tile([P, 4, 2, 2, P], bf16)
            nc.sync.dma_start(out=hb_sb[:], in_=hb[:])
            bt_sb = cpool.tile([P, 8], f32)
            nc.sync.dma_start(out=bt_sb[:], in_=bt[:])

            def emit_B(pend):
                sbt, S, zAr, zBr, o_sb = pend
                for h in range(2):
                    po = psO.tile([P, 512], f32)
                    nc.tensor.matmul(
                        po[:], lhsT=hb_sb[:, S, h, 0, :], rhs=zAr[:],
                        start=True, stop=False,
                    )
                    nc.tensor.matmul(
                        po[:], lhsT=hb_sb[:, S, h, 1, :], rhs=zBr[:],
                        start=False, stop=True,
                    )
                    c = 1024 * S + 512 * h
                    nc.scalar.add(
                        out=o_sb[:, c : c + 512],
                        in_=po[:],
                        add=bt_sb[:, 2 * S + h : 2 * S + h + 1],
                    )
                if S == 3:
                    nc.scalar.dma_start(out=out[sbt], in_=o_sb[:])

            for _rep in range(repeats):
                pend = None
                for sbt in range(SBT_PER_CORE):
                    xt_sb = xpool.tile([P, NCHUNK, 512], bf16)
                    nc.sync.dma_start(out=xt_sb[:], in_=xt[sbt])
                    o_sb = opool.tile([P, 4096], bf16)
                    for S in range(4):
                        zA = psA.tile([P, 512], f32, tag="zA")
                        zB = psA.tile([P, 512], f32, tag="zB")
                        for kk in range(4):
                            nc.tensor.matmul(
                                zA[32 * kk : 32 * kk + 32, :],
                                lhsT=ca_sb[:, kk, S, :],
                                rhs=xt_sb[:, kk, :],
                                start=True, stop=True,
                                tile_position=(0, 32 * kk),
                            )
                        for kk in range(4):
                            nc.tensor.matmul(
                                zB[32 * kk : 32 * kk + 32, :],
                                lhsT=ca_sb[:, 4 + kk, S, :],
                                rhs=xt_sb[:, 4 + kk, :],
                                start=True, stop=True,
                                tile_position=(0, 32 * kk),
                            )
                        zAr = zrp.tile([P, 512], bf16, tag="zAr")
                        nc.vector.tensor_copy(out=zAr[:], in_=zA[:])
                        zBr = zrp.tile([P, 512], bf16, tag="zBr")
                        nc.vector.tensor_copy(out=zBr[:], in_=zB[:])
                        if pend is not None:
                            emit_B(pend)
                        pend = (sbt, S, zAr, zBr, o_sb)
                emit_B(pend)
    nc.compile()
    return nc


def kernel_2lvl_v4c(x, twiddle, bias, _repeats=1):
    xt, ca, hb, bt = _pack_2lvl_v4(x, twiddle, bias)
    nc = _build_2lvl_v4c(repeats=_repeats, xtp_bufs=4, zrp_bufs=3, outp_bufs=3)
    in_maps = [
        {"xt": xt[k], "ca": ca, "hb": hb, "bt": bt} for k in range(N_CORES)
    ]
    res = run_bass_kernel_spmd(nc, in_maps, list(range(N_CORES)))
    return _unpack_2lvl_v4c([r["out"] for r in res.results])


# --- v4d: v4c but with per-S output DMAs ([128,1024] issued right after
# each S's bias-adds on the ACT ring) — earlier issue, smaller tail. ---


def _build_2lvl_v4d(repeats: int = 1) -> bass.Bass:
    nc = bacc.Bacc()
    f32 = mybir.dt.float32
    bf16 = mybir.dt.bfloat16

    xt = nc.declare_dram_parameter("xt", [SBT_PER_CORE, P, NCHUNK, 512], bf16, isOutput=False)
    ca = nc.declare_dram_parameter("ca", [P, 8, 4, 32], bf16, isOutput=False)
    hb = nc.declare_dram_parameter("hb", [P, 4, 2, 2, P], bf16, isOutput=False)
    bt = nc.declare_dram_parameter("bt", [P, 8], f32, isOutput=False)
    out = nc.declare_dram_parameter("out", [SBT_PER_CORE, 4, P, 1024], bf16, isOutput=True)

    with TileContext(nc) as tc:
        with (
            tc.tile_pool(name="const", bufs=1) as cpool,
            tc.tile_pool(name="xtp", bufs=3) as xpool,
            tc.tile_pool(name="zrp", bufs=2) as zrp,
            tc.tile_pool(name="outp", bufs=3) as opool,
            tc.tile_pool(name="psA", bufs=2, space="PSUM") as psA,
            tc.tile_pool(name="psO", bufs=4, space="PSUM") as psO,
        ):
            ca_sb = cpool.tile([P, 8, 4, 32], bf16)
            nc.sync.dma_start(out=ca_sb[:], in_=ca[:])
            hb_sb = cpool.tile([P, 4, 2, 2, P], bf16)
            nc.sync.dma_start(out=hb_sb[:], in_=hb[:])
            bt_sb = cpool.tile([P, 8], f32)
            nc.sync.dma_start(out=bt_sb[:], in_=bt[:])

            def emit_B(pend):
                sbt, S, zAr, zBr = pend
                o_sb = opool.tile([P, 1024], bf16)
                for h in range(2):
                    po = psO.tile([P, 512], f32)
                    nc.tensor.matmul(
                        po[:], lhsT=hb_sb[:, S, h, 0, :], rhs=zAr[:],
                        start=True, stop=False,
                    )
                    nc.tensor.matmul(
                        po[:], lhsT=hb_sb[:, S, h, 1, :], rhs=zBr[:],
                        start=False, stop=True,
                    )
                    nc.scalar.add(
                        out=o_sb[:, 512 * h : 512 * h + 512],
                        in_=po[:],
                        add=bt_sb[:, 2 * S + h : 2 * S + h + 1],
                    )
                nc.scalar.dma_start(out=out[sbt, S], in_=o_sb[:])

            for _rep in range(repeats):
                pend = None
                for sbt in range(SBT_PER_CORE):
                    xt_sb = xpool.tile([P, NCHUNK, 512], bf16)
                    nc.sync.dma_start(out=xt_sb[:], in_=xt[sbt])
                    for S in range(4):
                        zA = psA.tile([P, 512], f32, tag="zA")
                        zB = psA.tile([P, 512], f32, tag="zB")
                        for kk in range(4):
                            nc.tensor.matmul(
                                zA[32 * kk : 32 * kk + 32, :],
                                lhsT=ca_sb[:, kk, S, :],
                                rhs=xt_sb[:, kk, :],
                                start=True, stop=True,
                                tile_position=(0, 32 * kk),
                            )
                        for kk in range(4):
                            nc.tensor.matmul(
                                zB[32 * kk : 32 * kk + 32, :],
                                lhsT=ca_sb[:, 4 + kk, S, :],
                                rhs=xt_sb[:, 4 + kk, :],
                                start=True, stop=True,
                                tile_position=(0, 32 * kk),
                            )
                        zAr = zrp.tile([P, 512], bf16, tag="zAr")
                        nc.vector.tensor_copy(out=zAr[:], in_=zA[:])
                        zBr = zrp.tile([P, 512], bf16, tag="zBr")
                        nc.vector.tensor_copy(out=zBr[:], in_=zB[:])
                        if pend is not None:
                            emit_B(pend)
                        pend = (sbt, S, zAr, zBr)
                emit_B(pend)
    nc.compile()
    return nc


def kernel_2lvl_v4d(x, twiddle, bias, _repeats=1):
    xt, ca, hb, bt = _pack_2lvl_v4(x, twiddle, bias)
    nc = _build_2lvl_v4d(repeats=_repeats)
    in_maps = [
        {"xt": xt[k], "ca": ca, "hb": hb, "bt": bt} for k in range(N_CORES)
    ]
    res = run_bass_kernel_spmd(nc, in_maps, list(range(N_CORES)))
    return _unpack_2lvl_v4([r["out"] for r in res.results])


def _build_dma_probe(repeats: int = 1) -> bass.Bass:
    """Pure DMA floor probe: 8 MiB in + 8 MiB out bf16, no compute."""
    nc = bacc.Bacc()
    bf16 = mybir.dt.bfloat16
    xt = nc.declare_dram_parameter("xt", [SBT_PER_CORE, P, NCHUNK, 512], bf16, isOutput=False)
    out = nc.declare_dram_parameter(
        "out", [SBT_PER_CORE, 4, P, 1024], bf16, isOutput=True
    )
    with TileContext(nc) as tc:
        with tc.tile_pool(name="xtp", bufs=3) as xpool:
            for _rep in range(repeats):
                for sbt in range(SBT_PER_CORE):
                    xt_sb = xpool.tile([P, NCHUNK, 512], bf16)
                    nc.sync.dma_start(out=xt_sb[:], in_=xt[sbt])
                    for S in range(4):
                        src = xt_sb[:, 2 * S : 2 * S + 2, :].rearrange("p a b -> p (a b)")
                        nc.sync.dma_start(out=out[sbt, S], in_=src)
    nc.compile()
    return nc



